# revision 7
# baseline (speedup 1.0000x reference)
"""Trainium2 Bass kernel: MoE top-k router (top-8 of 64 experts + softmax).

Contract: kernel(logits, top_k) takes the FULL inputs (logits [1048576, 64]
f32, top_k == 8) and returns (topk_idx int64 [N, 8], topk_w f32 [N, 8]),
matching jax.lax.top_k + jax.nn.softmax semantics (stable descending order,
ties broken toward the smaller index).

Sharding: data-parallel over tokens across 8 NeuronCores (one SPMD program,
per-core slices fed via run_bass_kernel_spmd). Per core, tokens are laid out
partition-major — partition p owns tokens [p*1024, (p+1)*1024).

The top-8 selection runs as ONE hand-written custom DVE instruction per
[128, T, 64] tile (vs 3 stock match-unit instructions per 128 tokens): a
MAX8-style swap-flop MIN-cascade streams each token's 64 logits from SRC_0
(slice k's swap flop retains the (k+1)-th largest), then a FIND_INDEX8-style
IS_EQ match pass re-streams the same 64 SBUF words through the second read
port (SRC_1) latching each slice's match position, then 8 match indices and
8 values drain and the uOp chain loops to the next token. ~146 DVE cycles
per 128-token group instead of ~310. Indices drain as raw u32 bit patterns
into the f32 output tile; the host reinterprets them (match HW handles
duplicate values in jax tie order — verified on planted duplicates).

Softmax of the 8 selected values stays off the DVE: exp on ScalarE, the
denominator tree-sum and final scale on GPSIMD, reciprocal via the 1-cycle
DVE approx seed + Newton-Raphson refinement on GPSIMD (exact DVE reciprocal
on the final small tile to shorten the serial tail).
"""

import sys

if "/opt/trn_rl_repo" not in sys.path:
    sys.path.insert(0, "/opt/trn_rl_repo")

from dataclasses import dataclass

import numpy as np

N_TOKENS = 1048576
E = 64             # experts
K = 8              # top-k
NCORES = 8
P = 128            # SBUF partitions
TPC = N_TOKENS // NCORES   # tokens per core = 131072
TPP = TPC // P             # tokens per partition = 1024
T = 64                     # tokens per partition per full tile

_CACHE = {}


# --------------------------------------------------------------------------
# Custom fused top-8 DVE op (values + match indices in one instruction).
#
# uOp chain (intra-spec indices; tok_len = 64):
#   0: RAMP0 entry (elem 0 seeds stage-0 swap)   SRC_DONE->IDLE, CNT1->2
#   1: RAMP0 loop  (same config; loop target)    SRC_DONE->IDLE, CNT1->2
#   2..8: RAMP1..7 (MIN cascade j<k + seed k)    CNT1->next
#   9: STEADY_A    (8-stage MIN cascade)         CNT56->10
#  10: CLEAR       (clear_match bubble)          CNT1->11
#  11: STEADY_B    (IS_EQ(stream, swap), latch)  CNT64->12   [reads SRC_1]
#  12: SPACER      (pipeline flush bubble)       CNT1->13
#  13: IDX_DRAIN   (8x OutSel.MATCH_INDEX)       CNT8->14
#  14..21: VDRAIN s0..s7 (descending values)     CNT1->next; last->1 (loop)
# --------------------------------------------------------------------------

def _build_topk_uops(tok_len=64):
    from concourse.dve_uop import (
        AluInp, AluOp, InpSel, OutPath, OutSel, Trigger, UopConfig, ENABLE,
    )

    def ramp(k):
        u = UopConfig()
        u.enable_input(InpSel.SRC_0, 0)
        u.require_inp0 = ENABLE
        u.repeat_count = 1
        for j in range(k):
            u.datapath_config[j].enable_alu(
                AluOp.MIN, AluInp.CURR_SWAP_OUT, AluInp.PREV_ALU_OUT
            )
            u.datapath_config[j].swap_enable = ENABLE
        u.datapath_config[k].enable_alu(
            AluOp.BYPASS, AluInp.PREV_ALU_OUT, AluInp.PREV_ALU_OUT
        )
        u.datapath_config[k].swap_enable = ENABLE
        return u

    uops = []
    # Termination: src0's AP carries ONE extra element past the last token, so
    # ramp0 always has data to issue (it never stalls on requires_src0) and
    # the level-evaluated SRC_TENSOR_LT_8 fires on that issue cycle -> IDLE.
    # (Waiting stalled on SRC_TENSOR_DONE after the stream drained misses the
    # done event and leaves the uOp FSM stalled past instruction retirement,
    # wedging the engine for the next NEFF execution.)
    for _ in (0, 1):  # 0: entry, 1: loop re-entry (next_uop 0 means IDLE)
        u = ramp(0)
        u.trigger = (Trigger.SRC_TENSOR_LT_8, Trigger.COUNT, Trigger.NONE)
        u.next_uop = (0, 2, 0)
        uops.append(u)
    for k in range(1, 8):
        u = ramp(k)
        u.trigger = (Trigger.COUNT, Trigger.NONE, Trigger.NONE)
        u.next_uop = (k + 2, 0, 0)
        uops.append(u)

    u = UopConfig()  # 9: steady_A
    u.enable_input(InpSel.SRC_0, 0)
    u.require_inp0 = ENABLE
    u.repeat_count = tok_len - 8
    for j in range(8):
        u.datapath_config[j].enable_alu(
            AluOp.MIN, AluInp.PREV_ALU_OUT, AluInp.CURR_SWAP_OUT
        )
        u.datapath_config[j].swap_enable = ENABLE
    u.trigger = (Trigger.COUNT, Trigger.NONE, Trigger.NONE)
    u.next_uop = (10, 0, 0)
    uops.append(u)

    u = UopConfig()  # 10: clear_match bubble
    u.repeat_count = 1
    u.clear_match = ENABLE
    u.trigger = (Trigger.COUNT, Trigger.NONE, Trigger.NONE)
    u.next_uop = (11, 0, 0)
    uops.append(u)

    u = UopConfig()  # 11: steady_B — match pass over the second read port
    u.enable_input(InpSel.SRC_1, 1)  # lane 1 -> delay chain 0
    u.require_inp1 = ENABLE
    u.repeat_count = tok_len
    u.valid_match = ENABLE
    for j in range(8):
        u.datapath_config[j].enable_alu(
            AluOp.IS_EQ, AluInp.PREV_DELAY_0, AluInp.CURR_SWAP_OUT
        )
        u.datapath_config[j].pass_through_delay(0)
    u.trigger = (Trigger.COUNT, Trigger.NONE, Trigger.NONE)
    u.next_uop = (12, 0, 0)
    uops.append(u)

    u = UopConfig()  # 12: spacer
    u.repeat_count = 1
    u.trigger = (Trigger.COUNT, Trigger.NONE, Trigger.NONE)
    u.next_uop = (13, 0, 0)
    uops.append(u)

    u = UopConfig()  # 13: idx drain
    u.repeat_count = 8
    u.enable_output(OutSel.MATCH_INDEX, OutPath.WR0_LO)
    u.trigger = (Trigger.COUNT, Trigger.NONE, Trigger.NONE)
    u.next_uop = (14, 0, 0)
    uops.append(u)

    for k in range(8):  # 14..21: value drains, slice 0 (largest) first
        u = UopConfig()
        u.repeat_count = 1
        u.datapath_config[k].enable_alu(
            AluOp.BYPASS, AluInp.CURR_SWAP_OUT, AluInp.CURR_SWAP_OUT
        )
        for j in range(k + 1, 8):
            u.datapath_config[j].pass_through_alu()
        u.enable_output(OutSel.ALU_OUT, OutPath.WR0_LO)
        u.trigger = (Trigger.COUNT, Trigger.NONE, Trigger.NONE)
        u.next_uop = (15 + k if k < 7 else 1, 0, 0)
        uops.append(u)
    return uops


def _get_topk_op():
    if "op" in _CACHE:
        return _CACHE["op"]
    from concourse.dve_ops import (
        DveOp, OPS, CUSTOM_DVE_SPECS, _SUB_OPCODE_FOR_NAME, get_dve_sub_opcode,
    )
    from concourse.dve_spec import Spec, Src0, Src1
    from concourse.dve_uop import DveOpSpec

    uops = _build_topk_uops(E)
    # op name carries the uop-bytes hash: a uop edit changes the BIR and so
    # the NEFF cache key, preventing stale-table reuse.
    tag = DveOpSpec(name="probe", opcode=1, uops=uops, rd1_en=True).sha("v3")[:8]
    name = f"TOPK8_{tag}"

    @dataclass(frozen=True)
    class RawDveOp(DveOp):
        raw_uops: tuple = ()

        def compile(self, ver):
            assert ver == "v3", f"hand-written for TRN2/v3 only, got {ver}"
            return DveOpSpec(
                name=self.name,
                opcode=get_dve_sub_opcode(self.name),
                uops=list(self.raw_uops),
                rd1_en=True,
            )

    def _ref(in0, in1, s0, s1, imm2):
        # CoreSim-only; the HW path never calls this.
        p = in0.shape[0]
        x = in0.reshape(p, -1, E)
        t = x.shape[1]
        out = np.zeros((p, t * 16), dtype=np.float32)
        order = np.argsort(-x, axis=-1, kind="stable")[..., :8]
        vals = np.take_along_axis(x, order, axis=-1)
        out.reshape(p, t, 16)[:, :, 0:8] = order.astype(np.uint32).view(np.float32)
        out.reshape(p, t, 16)[:, :, 8:16] = vals
        return out

    op = RawDveOp(
        name=name,
        spec=Spec(body=Src0 + Src1, reference=_ref),
        subdim=False,
        uops_sha={},
        raw_uops=tuple(uops),
    )
    if name not in _SUB_OPCODE_FOR_NAME:
        row = max(_SUB_OPCODE_FOR_NAME.values()) + 1
        assert row < 0x20, f"row {row} overflows the 5-bit byte-36 field"
        OPS.append(op)
        CUSTOM_DVE_SPECS[op.name] = op.spec
        _SUB_OPCODE_FOR_NAME[op.name] = row
    _CACHE["op"] = op
    return op


def _build(tpp=TPP, t_tile=T):
    import concourse.bacc as bacc
    import concourse.mybir as mybir
    import concourse.tile as tile

    f32 = mybir.dt.float32
    op = _get_topk_op()

    n_tok = P * tpp
    # small first tiles (DVE starts after ~256KB of DMA instead of 2MB) and a
    # small last tile (short softmax/store tail after the final DVE instr)
    if tpp == 1024 and t_tile == 64:
        sizes = [8, 56] + [64] * 14 + [48, 16]
    else:
        sizes = [t_tile] * (tpp // t_tile)
    assert sum(sizes) == tpp
    offs = [sum(sizes[:j]) for j in range(len(sizes))]

    nc = bacc.Bacc("TRN2", target_bir_lowering=False, debug=False)
    logits = nc.dram_tensor("logits", [n_tok, E], f32, kind="ExternalInput")
    # y_out row = [idx0..idx7 (u32 bit patterns), v0..v7]; DMAing the whole
    # tile keeps the store contiguous (one descriptor per partition) — the
    # host reads the idx half and ignores the value half. w is the softmax.
    y_out = nc.dram_tensor("y_out", [n_tok, 16], f32, kind="ExternalOutput")
    w_out = nc.dram_tensor("w_out", [n_tok, K], f32, kind="ExternalOutput")

    lg_v = logits.ap().rearrange("(p t) e -> p t e", p=P, t=tpp)
    y_v = y_out.ap().rearrange("(p t) k -> p t k", p=P, t=tpp)
    w_v = w_out.ap().rearrange("(p t) k -> p t k", p=P, t=tpp)

    with tile.TileContext(nc) as tc:
        with tc.tile_pool(name="io", bufs=4) as pool:
            for o, tt in zip(offs, sizes):
                # one extra trailing element feeds the LT_8 termination issue
                x = pool.tile([P, tt * E + 1], f32, tag="x")
                x3 = x[:, 0:tt * E].rearrange("p (t e) -> p t e", t=tt, e=E)
                nc.sync.dma_start(x3, lg_v[:, o:o + tt, :])
                y = pool.tile([P, tt, 16], f32, tag="y")
                nc.vector._custom_dve(op, out=y[:], in0=x[:], in1=x3,
                                      s0=0.0, s1=0.0)
                vals = y[:, :, 8:16]
                ex = pool.tile([P, tt, K], f32, tag="ex")
                nc.scalar.activation(
                    ex[:], vals, mybir.ActivationFunctionType.Exp
                )
                # softmax denominator: pairwise tree-sum on GPSIMD
                t1 = pool.tile([P, tt, 4], f32, tag="t1")
                t2 = pool.tile([P, tt, 2], f32, tag="t2")
                s = pool.tile([P, tt, 1], f32, tag="s")
                nc.gpsimd.tensor_add(t1[:], ex[:, :, 0:4], ex[:, :, 4:8])
                nc.gpsimd.tensor_add(t2[:], t1[:, :, 0:2], t1[:, :, 2:4])
                nc.gpsimd.tensor_add(s[:], t2[:, :, 0:1], t2[:, :, 1:2])
                # exact DVE reciprocal: ~127ns/tile on the DVE, and it drops
                # the 3-op GPSIMD Newton-Raphson chain (GPSIMD per-instruction
                # overhead ~1us dominates tiny refinement ops)
                r = pool.tile([P, tt, 1], f32, tag="r")
                nc.vector.reciprocal(r[:], s[:])
                w = pool.tile([P, tt, K], f32, tag="w")
                nc.gpsimd.tensor_mul(w[:], ex[:], r[:].broadcast_to([P, tt, K]))
                nc.sync.dma_start(y_v[:, o:o + tt, :], y[:])
                nc.sync.dma_start(w_v[:, o:o + tt, :], w[:])
    nc.compile()
    return nc


def _get_nc():
    if "nc" not in _CACHE:
        _CACHE["nc"] = _build()
    return _CACHE["nc"]


def kernel(logits, top_k):
    logits = np.asarray(logits, dtype=np.float32)
    k = int(np.asarray(top_k))
    assert k == K, f"kernel hardcodes top_k={K}, got {k}"
    assert logits.shape == (N_TOKENS, E), logits.shape

    from concourse.bass_utils import run_bass_kernel_spmd

    nc = _get_nc()
    chunks = logits.reshape(NCORES, TPC, E)
    in_maps = [{"logits": np.ascontiguousarray(chunks[c])} for c in range(NCORES)]
    # The tunneled devices occasionally fail a run with a transient
    # NRT_EXEC_UNIT_UNRECOVERABLE error; a straight retry recovers.
    last_err = None
    for _attempt in range(3):
        try:
            res = run_bass_kernel_spmd(nc, in_maps, list(range(NCORES)))
            break
        except Exception as e:  # noqa: BLE001 - retry transient device faults
            last_err = e
            import time as _time

            _time.sleep(5.0)
    else:
        raise last_err

    # Row r of each per-core output is token r of that core's slice, so a
    # plain concat along the token axis reassembles the full outputs.
    y = np.concatenate([r["y_out"] for r in res.results], axis=0)
    w = np.concatenate([r["w_out"] for r in res.results], axis=0)
    idx = np.ascontiguousarray(y[:, 0:K]).view(np.uint32).astype(np.int64)
    return idx, w.astype(np.float32)


# revision 8
# speedup vs baseline: 1.0865x; 1.0865x over previous
"""Trainium2 Bass kernel: MoE top-k router (top-8 of 64 experts + softmax).

Contract: kernel(logits, top_k) takes the FULL inputs (logits [1048576, 64]
f32, top_k == 8) and returns (topk_idx int64 [N, 8], topk_w f32 [N, 8]),
matching jax.lax.top_k + jax.nn.softmax semantics (stable descending order,
ties broken toward the smaller index).

Sharding: data-parallel over tokens across 8 NeuronCores (one SPMD program,
per-core slices fed via run_bass_kernel_spmd). Per core, tokens are laid out
partition-major — partition p owns tokens [p*1024, (p+1)*1024).

The top-8 selection runs as ONE hand-written custom DVE instruction per
[128, T, 64] tile (vs 3 stock match-unit instructions per 128 tokens): a
MAX8-style swap-flop MIN-cascade streams each token's 64 logits from SRC_0
(slice k's swap flop retains the (k+1)-th largest), then a FIND_INDEX8-style
IS_EQ match pass re-streams the same 64 SBUF words through the second read
port (SRC_1) latching each slice's match position, then 8 match indices and
8 values drain and the uOp chain loops to the next token. ~146 DVE cycles
per 128-token group instead of ~310. Indices drain as raw u32 bit patterns
into the f32 output tile; the host reinterprets them (match HW handles
duplicate values in jax tie order — verified on planted duplicates).

Softmax of the 8 selected values stays off the DVE: exp on ScalarE, the
denominator tree-sum and final scale on GPSIMD, reciprocal via the 1-cycle
DVE approx seed + Newton-Raphson refinement on GPSIMD (exact DVE reciprocal
on the final small tile to shorten the serial tail).
"""

import sys

if "/opt/trn_rl_repo" not in sys.path:
    sys.path.insert(0, "/opt/trn_rl_repo")

from dataclasses import dataclass

import numpy as np

N_TOKENS = 1048576
E = 64             # experts
K = 8              # top-k
NCORES = 8
P = 128            # SBUF partitions
TPC = N_TOKENS // NCORES   # tokens per core = 131072
TPP = TPC // P             # tokens per partition = 1024
T = 64                     # tokens per partition per full tile

_CACHE = {}


# --------------------------------------------------------------------------
# Custom fused top-8 DVE op (values + match indices in one instruction).
#
# uOp chain (intra-spec indices; tok_len = 64):
#   0: RAMP0 entry (elem 0 seeds stage-0 swap)   SRC_DONE->IDLE, CNT1->2
#   1: RAMP0 loop  (same config; loop target)    SRC_DONE->IDLE, CNT1->2
#   2..8: RAMP1..7 (MIN cascade j<k + seed k)    CNT1->next
#   9: STEADY_A    (8-stage MIN cascade)         CNT56->10
#  10: CLEAR       (clear_match bubble)          CNT1->11
#  11: STEADY_B    (IS_EQ(stream, swap), latch)  CNT64->12   [reads SRC_1]
#  12: SPACER      (pipeline flush bubble)       CNT1->13
#  13: IDX_DRAIN   (8x OutSel.MATCH_INDEX)       CNT8->14
#  14..21: VDRAIN s0..s7 (descending values)     CNT1->next; last->1 (loop)
# --------------------------------------------------------------------------

def _build_topk_uops(tok_len=64):
    from concourse.dve_uop import (
        AluInp, AluOp, InpSel, OutPath, OutSel, Trigger, UopConfig, ENABLE,
    )

    def ramp(k):
        u = UopConfig()
        u.enable_input(InpSel.SRC_0, 0)
        u.require_inp0 = ENABLE
        u.repeat_count = 1
        for j in range(k):
            u.datapath_config[j].enable_alu(
                AluOp.MIN, AluInp.CURR_SWAP_OUT, AluInp.PREV_ALU_OUT
            )
            u.datapath_config[j].swap_enable = ENABLE
        u.datapath_config[k].enable_alu(
            AluOp.BYPASS, AluInp.PREV_ALU_OUT, AluInp.PREV_ALU_OUT
        )
        u.datapath_config[k].swap_enable = ENABLE
        return u

    uops = []
    # Termination: src0's AP carries ONE extra element past the last token, so
    # ramp0 always has data to issue (it never stalls on requires_src0) and
    # the level-evaluated SRC_TENSOR_LT_8 fires on that issue cycle -> IDLE.
    # (Waiting stalled on SRC_TENSOR_DONE after the stream drained misses the
    # done event and leaves the uOp FSM stalled past instruction retirement,
    # wedging the engine for the next NEFF execution.)
    for _ in (0, 1):  # 0: entry, 1: loop re-entry (next_uop 0 means IDLE)
        u = ramp(0)
        u.trigger = (Trigger.SRC_TENSOR_LT_8, Trigger.COUNT, Trigger.NONE)
        u.next_uop = (0, 2, 0)
        uops.append(u)
    for k in range(1, 8):
        u = ramp(k)
        u.trigger = (Trigger.COUNT, Trigger.NONE, Trigger.NONE)
        u.next_uop = (k + 2, 0, 0)
        uops.append(u)

    u = UopConfig()  # 9: steady_A
    u.enable_input(InpSel.SRC_0, 0)
    u.require_inp0 = ENABLE
    u.repeat_count = tok_len - 8
    for j in range(8):
        u.datapath_config[j].enable_alu(
            AluOp.MIN, AluInp.PREV_ALU_OUT, AluInp.CURR_SWAP_OUT
        )
        u.datapath_config[j].swap_enable = ENABLE
    u.trigger = (Trigger.COUNT, Trigger.NONE, Trigger.NONE)
    u.next_uop = (10, 0, 0)
    uops.append(u)

    u = UopConfig()  # 10: clear_match bubble
    u.repeat_count = 1
    u.clear_match = ENABLE
    u.trigger = (Trigger.COUNT, Trigger.NONE, Trigger.NONE)
    u.next_uop = (11, 0, 0)
    uops.append(u)

    u = UopConfig()  # 11: steady_B — match pass over the second read port
    u.enable_input(InpSel.SRC_1, 1)  # lane 1 -> delay chain 0
    u.require_inp1 = ENABLE
    u.repeat_count = tok_len
    u.valid_match = ENABLE
    for j in range(8):
        u.datapath_config[j].enable_alu(
            AluOp.IS_EQ, AluInp.PREV_DELAY_0, AluInp.CURR_SWAP_OUT
        )
        u.datapath_config[j].pass_through_delay(0)
    u.trigger = (Trigger.COUNT, Trigger.NONE, Trigger.NONE)
    u.next_uop = (12, 0, 0)
    uops.append(u)

    u = UopConfig()  # 12: spacer
    u.repeat_count = 1
    u.trigger = (Trigger.COUNT, Trigger.NONE, Trigger.NONE)
    u.next_uop = (13, 0, 0)
    uops.append(u)

    u = UopConfig()  # 13: idx drain
    u.repeat_count = 8
    u.enable_output(OutSel.MATCH_INDEX, OutPath.WR0_LO)
    u.trigger = (Trigger.COUNT, Trigger.NONE, Trigger.NONE)
    u.next_uop = (14, 0, 0)
    uops.append(u)

    for k in range(8):  # 14..21: value drains, slice 0 (largest) first
        u = UopConfig()
        u.repeat_count = 1
        u.datapath_config[k].enable_alu(
            AluOp.BYPASS, AluInp.CURR_SWAP_OUT, AluInp.CURR_SWAP_OUT
        )
        for j in range(k + 1, 8):
            u.datapath_config[j].pass_through_alu()
        u.enable_output(OutSel.ALU_OUT, OutPath.WR0_LO)
        u.trigger = (Trigger.COUNT, Trigger.NONE, Trigger.NONE)
        u.next_uop = (15 + k if k < 7 else 1, 0, 0)
        uops.append(u)
    return uops


def _get_topk_op():
    if "op" in _CACHE:
        return _CACHE["op"]
    from concourse.dve_ops import (
        DveOp, OPS, CUSTOM_DVE_SPECS, _SUB_OPCODE_FOR_NAME, get_dve_sub_opcode,
    )
    from concourse.dve_spec import Spec, Src0, Src1
    from concourse.dve_uop import DveOpSpec

    uops = _build_topk_uops(E)
    # op name carries the uop-bytes hash: a uop edit changes the BIR and so
    # the NEFF cache key, preventing stale-table reuse.
    tag = DveOpSpec(name="probe", opcode=1, uops=uops, rd1_en=True).sha("v3")[:8]
    name = f"TOPK8_{tag}"

    @dataclass(frozen=True)
    class RawDveOp(DveOp):
        raw_uops: tuple = ()

        def compile(self, ver):
            assert ver == "v3", f"hand-written for TRN2/v3 only, got {ver}"
            return DveOpSpec(
                name=self.name,
                opcode=get_dve_sub_opcode(self.name),
                uops=list(self.raw_uops),
                rd1_en=True,
            )

    def _ref(in0, in1, s0, s1, imm2):
        # CoreSim-only; the HW path never calls this.
        p = in0.shape[0]
        x = in0.reshape(p, -1, E)
        t = x.shape[1]
        out = np.zeros((p, t * 16), dtype=np.float32)
        order = np.argsort(-x, axis=-1, kind="stable")[..., :8]
        vals = np.take_along_axis(x, order, axis=-1)
        out.reshape(p, t, 16)[:, :, 0:8] = order.astype(np.uint32).view(np.float32)
        out.reshape(p, t, 16)[:, :, 8:16] = vals
        return out

    op = RawDveOp(
        name=name,
        spec=Spec(body=Src0 + Src1, reference=_ref),
        subdim=False,
        uops_sha={},
        raw_uops=tuple(uops),
    )
    if name not in _SUB_OPCODE_FOR_NAME:
        row = max(_SUB_OPCODE_FOR_NAME.values()) + 1
        assert row < 0x20, f"row {row} overflows the 5-bit byte-36 field"
        OPS.append(op)
        CUSTOM_DVE_SPECS[op.name] = op.spec
        _SUB_OPCODE_FOR_NAME[op.name] = row
    _CACHE["op"] = op
    return op


def _build(tpp=TPP, t_tile=T):
    import concourse.bacc as bacc
    import concourse.mybir as mybir
    import concourse.tile as tile

    f32 = mybir.dt.float32
    op = _get_topk_op()

    n_tok = P * tpp
    # small first tiles (DVE starts after ~256KB of DMA instead of 2MB) and a
    # small last tile (short softmax/store tail after the final DVE instr)
    if tpp == 1024 and t_tile == 64:
        sizes = [8, 56] + [64] * 14 + [48, 16]
    else:
        sizes = [t_tile] * (tpp // t_tile)
    assert sum(sizes) == tpp
    offs = [sum(sizes[:j]) for j in range(len(sizes))]

    nc = bacc.Bacc("TRN2", target_bir_lowering=False, debug=False)
    logits = nc.dram_tensor("logits", [n_tok, E], f32, kind="ExternalInput")
    # y_out row = [idx0..idx7 (u32 bit patterns), v0..v7]; DMAing the whole
    # tile keeps the store contiguous (one descriptor per partition) — the
    # host reads the idx half and ignores the value half. w is the softmax.
    y_out = nc.dram_tensor("y_out", [n_tok, 16], f32, kind="ExternalOutput")
    w_out = nc.dram_tensor("w_out", [n_tok, K], f32, kind="ExternalOutput")

    lg_v = logits.ap().rearrange("(p t) e -> p t e", p=P, t=tpp)
    y_v = y_out.ap().rearrange("(p t) k -> p t k", p=P, t=tpp)
    w_v = w_out.ap().rearrange("(p t) k -> p t k", p=P, t=tpp)

    with tile.TileContext(nc) as tc:
        with tc.tile_pool(name="io", bufs=4) as pool:
            for o, tt in zip(offs, sizes):
                # one extra trailing element feeds the LT_8 termination issue
                x = pool.tile([P, tt * E + 1], f32, tag="x")
                x3 = x[:, 0:tt * E].rearrange("p (t e) -> p t e", t=tt, e=E)
                nc.sync.dma_start(x3, lg_v[:, o:o + tt, :])
                y = pool.tile([P, tt, 16], f32, tag="y")
                nc.vector._custom_dve(op, out=y[:], in0=x[:], in1=x3,
                                      s0=0.0, s1=0.0)
                vals = y[:, :, 8:16]
                ex = pool.tile([P, tt, K], f32, tag="ex")
                nc.scalar.activation(
                    ex[:], vals, mybir.ActivationFunctionType.Exp
                )
                # softmax denominator: pairwise tree-sum on GPSIMD
                t1 = pool.tile([P, tt, 4], f32, tag="t1")
                t2 = pool.tile([P, tt, 2], f32, tag="t2")
                s = pool.tile([P, tt, 1], f32, tag="s")
                nc.gpsimd.tensor_add(t1[:], ex[:, :, 0:4], ex[:, :, 4:8])
                nc.gpsimd.tensor_add(t2[:], t1[:, :, 0:2], t1[:, :, 2:4])
                nc.gpsimd.tensor_add(s[:], t2[:, :, 0:1], t2[:, :, 1:2])
                # 1/s = exp(-ln(s)) on the Act engine (exp and ln share one
                # activation table set, so no per-tile table reloads). Keeps
                # the DVE queue free for the fused top-8 instructions and
                # avoids the slow small-tensor GPSIMD Newton-Raphson chain.
                ls = pool.tile([P, tt, 1], f32, tag="ls")
                nc.scalar.activation(ls[:], s[:],
                                     mybir.ActivationFunctionType.Ln)
                r = pool.tile([P, tt, 1], f32, tag="r")
                nc.scalar.activation(r[:], ls[:],
                                     mybir.ActivationFunctionType.Exp,
                                     scale=-1.0)
                w = pool.tile([P, tt, K], f32, tag="w")
                nc.gpsimd.tensor_mul(w[:], ex[:], r[:].broadcast_to([P, tt, K]))
                # output DMAs issue from the Pool queue: the serial Sync queue
                # then carries ONLY input loads, so the next tile's x transfer
                # is never queued behind output DMAs that wait on the softmax
                # chain (that serialization cost ~7us/tile of DVE idle).
                nc.gpsimd.dma_start(y_v[:, o:o + tt, :], y[:])
                nc.gpsimd.dma_start(w_v[:, o:o + tt, :], w[:])
    nc.compile()
    return nc


def _get_nc():
    if "nc" not in _CACHE:
        _CACHE["nc"] = _build()
    return _CACHE["nc"]


def kernel(logits, top_k):
    logits = np.asarray(logits, dtype=np.float32)
    k = int(np.asarray(top_k))
    assert k == K, f"kernel hardcodes top_k={K}, got {k}"
    assert logits.shape == (N_TOKENS, E), logits.shape

    from concourse.bass_utils import run_bass_kernel_spmd

    nc = _get_nc()
    chunks = logits.reshape(NCORES, TPC, E)
    in_maps = [{"logits": np.ascontiguousarray(chunks[c])} for c in range(NCORES)]
    # The tunneled devices occasionally fail a run with a transient
    # NRT_EXEC_UNIT_UNRECOVERABLE error; a straight retry recovers.
    last_err = None
    for _attempt in range(3):
        try:
            res = run_bass_kernel_spmd(nc, in_maps, list(range(NCORES)))
            break
        except Exception as e:  # noqa: BLE001 - retry transient device faults
            last_err = e
            import time as _time

            _time.sleep(5.0)
    else:
        raise last_err

    # Row r of each per-core output is token r of that core's slice, so a
    # plain concat along the token axis reassembles the full outputs.
    y = np.concatenate([r["y_out"] for r in res.results], axis=0)
    w = np.concatenate([r["w_out"] for r in res.results], axis=0)
    idx = np.ascontiguousarray(y[:, 0:K]).view(np.uint32).astype(np.int64)
    return idx, w.astype(np.float32)


# revision 9
# speedup vs baseline: 1.3596x; 1.2514x over previous
"""Trainium2 Bass kernel: MoE top-k router (top-8 of 64 experts + softmax).

Contract: kernel(logits, top_k) takes the FULL inputs (logits [1048576, 64]
f32, top_k == 8) and returns (topk_idx int64 [N, 8], topk_w f32 [N, 8]),
matching jax.lax.top_k + jax.nn.softmax semantics (stable descending order,
ties broken toward the smaller index).

Sharding: data-parallel over tokens across 8 NeuronCores (one SPMD program,
per-core slices fed via run_bass_kernel_spmd). Per core, tokens are laid out
partition-major — partition p owns tokens [p*1024, (p+1)*1024).

The top-8 selection runs as ONE hand-written custom DVE instruction per
[128, T, 64] tile (vs 3 stock match-unit instructions per 128 tokens): a
MAX8-style swap-flop MIN-cascade streams each token's 64 logits from SRC_0
(slice k's swap flop retains the (k+1)-th largest), then a FIND_INDEX8-style
IS_EQ match pass re-streams the same 64 SBUF words through the second read
port (SRC_1) latching each slice's match position, then 8 match indices and
8 values drain and the uOp chain loops to the next token. ~146 DVE cycles
per 128-token group instead of ~310. Indices drain as raw u32 bit patterns
into the f32 output tile; the host reinterprets them (match HW handles
duplicate values in jax tie order — verified on planted duplicates).

Softmax of the 8 selected values stays off the DVE: exp on ScalarE, the
denominator tree-sum and final scale on GPSIMD, reciprocal via the 1-cycle
DVE approx seed + Newton-Raphson refinement on GPSIMD (exact DVE reciprocal
on the final small tile to shorten the serial tail).
"""

import sys

if "/opt/trn_rl_repo" not in sys.path:
    sys.path.insert(0, "/opt/trn_rl_repo")

from dataclasses import dataclass

import numpy as np

N_TOKENS = 1048576
E = 64             # experts
K = 8              # top-k
NCORES = 8
P = 128            # SBUF partitions
TPC = N_TOKENS // NCORES   # tokens per core = 131072
TPP = TPC // P             # tokens per partition = 1024
T = 64                     # tokens per partition per full tile

_CACHE = {}


# --------------------------------------------------------------------------
# Custom fused top-8 DVE op (values + match indices in one instruction).
#
# uOp chain (intra-spec indices; tok_len = 64):
#   0: RAMP0 entry (elem 0 seeds stage-0 swap)   SRC_DONE->IDLE, CNT1->2
#   1: RAMP0 loop  (same config; loop target)    SRC_DONE->IDLE, CNT1->2
#   2..8: RAMP1..7 (MIN cascade j<k + seed k)    CNT1->next
#   9: STEADY_A    (8-stage MIN cascade)         CNT56->10
#  10: CLEAR       (clear_match bubble)          CNT1->11
#  11: STEADY_B    (IS_EQ(stream, swap), latch)  CNT64->12   [reads SRC_1]
#  12: SPACER      (pipeline flush bubble)       CNT1->13
#  13: IDX_DRAIN   (8x OutSel.MATCH_INDEX)       CNT8->14
#  14..21: VDRAIN s0..s7 (descending values)     CNT1->next; last->1 (loop)
# --------------------------------------------------------------------------

def _build_topk_uops(tok_len=64):
    from concourse.dve_uop import (
        AluInp, AluOp, InpSel, OutPath, OutSel, Trigger, UopConfig, ENABLE,
    )

    def ramp(k):
        u = UopConfig()
        u.enable_input(InpSel.SRC_0, 0)
        u.require_inp0 = ENABLE
        u.repeat_count = 1
        for j in range(k):
            u.datapath_config[j].enable_alu(
                AluOp.MIN, AluInp.CURR_SWAP_OUT, AluInp.PREV_ALU_OUT
            )
            u.datapath_config[j].swap_enable = ENABLE
        u.datapath_config[k].enable_alu(
            AluOp.BYPASS, AluInp.PREV_ALU_OUT, AluInp.PREV_ALU_OUT
        )
        u.datapath_config[k].swap_enable = ENABLE
        return u

    uops = []
    # Termination: src0's AP carries ONE extra element past the last token, so
    # ramp0 always has data to issue (it never stalls on requires_src0) and
    # the level-evaluated SRC_TENSOR_LT_8 fires on that issue cycle -> IDLE.
    # (Waiting stalled on SRC_TENSOR_DONE after the stream drained misses the
    # done event and leaves the uOp FSM stalled past instruction retirement,
    # wedging the engine for the next NEFF execution.)
    for _ in (0, 1):  # 0: entry, 1: loop re-entry (next_uop 0 means IDLE)
        u = ramp(0)
        u.trigger = (Trigger.SRC_TENSOR_LT_8, Trigger.COUNT, Trigger.NONE)
        u.next_uop = (0, 2, 0)
        uops.append(u)
    for k in range(1, 8):
        u = ramp(k)
        u.trigger = (Trigger.COUNT, Trigger.NONE, Trigger.NONE)
        u.next_uop = (k + 2, 0, 0)
        uops.append(u)

    u = UopConfig()  # 9: steady_A
    u.enable_input(InpSel.SRC_0, 0)
    u.require_inp0 = ENABLE
    u.repeat_count = tok_len - 8
    for j in range(8):
        u.datapath_config[j].enable_alu(
            AluOp.MIN, AluInp.PREV_ALU_OUT, AluInp.CURR_SWAP_OUT
        )
        u.datapath_config[j].swap_enable = ENABLE
    u.trigger = (Trigger.COUNT, Trigger.NONE, Trigger.NONE)
    u.next_uop = (10, 0, 0)
    uops.append(u)

    u = UopConfig()  # 10: clear_match bubble
    u.repeat_count = 1
    u.clear_match = ENABLE
    u.trigger = (Trigger.COUNT, Trigger.NONE, Trigger.NONE)
    u.next_uop = (11, 0, 0)
    uops.append(u)

    u = UopConfig()  # 11: steady_B — match pass over the second read port
    u.enable_input(InpSel.SRC_1, 1)  # lane 1 -> delay chain 0
    u.require_inp1 = ENABLE
    u.repeat_count = tok_len
    u.valid_match = ENABLE
    for j in range(8):
        u.datapath_config[j].enable_alu(
            AluOp.IS_EQ, AluInp.PREV_DELAY_0, AluInp.CURR_SWAP_OUT
        )
        u.datapath_config[j].pass_through_delay(0)
    u.trigger = (Trigger.COUNT, Trigger.NONE, Trigger.NONE)
    u.next_uop = (12, 0, 0)
    uops.append(u)

    u = UopConfig()  # 12: spacer
    u.repeat_count = 1
    u.trigger = (Trigger.COUNT, Trigger.NONE, Trigger.NONE)
    u.next_uop = (13, 0, 0)
    uops.append(u)

    u = UopConfig()  # 13: idx drain
    u.repeat_count = 8
    u.enable_output(OutSel.MATCH_INDEX, OutPath.WR0_LO)
    u.trigger = (Trigger.COUNT, Trigger.NONE, Trigger.NONE)
    u.next_uop = (14, 0, 0)
    uops.append(u)

    for k in range(8):  # 14..21: value drains, slice 0 (largest) first
        u = UopConfig()
        u.repeat_count = 1
        u.datapath_config[k].enable_alu(
            AluOp.BYPASS, AluInp.CURR_SWAP_OUT, AluInp.CURR_SWAP_OUT
        )
        for j in range(k + 1, 8):
            u.datapath_config[j].pass_through_alu()
        u.enable_output(OutSel.ALU_OUT, OutPath.WR0_LO)
        u.trigger = (Trigger.COUNT, Trigger.NONE, Trigger.NONE)
        u.next_uop = (15 + k if k < 7 else 1, 0, 0)
        uops.append(u)
    return uops


def _get_topk_op():
    if "op" in _CACHE:
        return _CACHE["op"]
    from concourse.dve_ops import (
        DveOp, OPS, CUSTOM_DVE_SPECS, _SUB_OPCODE_FOR_NAME, get_dve_sub_opcode,
    )
    from concourse.dve_spec import Spec, Src0, Src1
    from concourse.dve_uop import DveOpSpec

    uops = _build_topk_uops(E)
    # op name carries the uop-bytes hash: a uop edit changes the BIR and so
    # the NEFF cache key, preventing stale-table reuse.
    tag = DveOpSpec(name="probe", opcode=1, uops=uops, rd1_en=True).sha("v3")[:8]
    name = f"TOPK8_{tag}"

    @dataclass(frozen=True)
    class RawDveOp(DveOp):
        raw_uops: tuple = ()

        def compile(self, ver):
            assert ver == "v3", f"hand-written for TRN2/v3 only, got {ver}"
            return DveOpSpec(
                name=self.name,
                opcode=get_dve_sub_opcode(self.name),
                uops=list(self.raw_uops),
                rd1_en=True,
            )

    def _ref(in0, in1, s0, s1, imm2):
        # CoreSim-only; the HW path never calls this.
        p = in0.shape[0]
        x = in0.reshape(p, -1, E)
        t = x.shape[1]
        out = np.zeros((p, t * 16), dtype=np.float32)
        order = np.argsort(-x, axis=-1, kind="stable")[..., :8]
        vals = np.take_along_axis(x, order, axis=-1)
        out.reshape(p, t, 16)[:, :, 0:8] = order.astype(np.uint32).view(np.float32)
        out.reshape(p, t, 16)[:, :, 8:16] = vals
        return out

    op = RawDveOp(
        name=name,
        spec=Spec(body=Src0 + Src1, reference=_ref),
        subdim=False,
        uops_sha={},
        raw_uops=tuple(uops),
    )
    if name not in _SUB_OPCODE_FOR_NAME:
        row = max(_SUB_OPCODE_FOR_NAME.values()) + 1
        assert row < 0x20, f"row {row} overflows the 5-bit byte-36 field"
        OPS.append(op)
        CUSTOM_DVE_SPECS[op.name] = op.spec
        _SUB_OPCODE_FOR_NAME[op.name] = row
    _CACHE["op"] = op
    return op


def _build(tpp=TPP, t_tile=T):
    import concourse.bacc as bacc
    import concourse.mybir as mybir
    import concourse.tile as tile

    f32 = mybir.dt.float32
    op = _get_topk_op()

    n_tok = P * tpp
    # small first tiles (DVE starts after ~256KB of DMA instead of 2MB) and a
    # small last tile (short softmax/store tail after the final DVE instr)
    if tpp == 1024 and t_tile == 64:
        sizes = [8, 56] + [64] * 14 + [48, 16]
    else:
        sizes = [t_tile] * (tpp // t_tile)
    assert sum(sizes) == tpp
    offs = [sum(sizes[:j]) for j in range(len(sizes))]

    nc = bacc.Bacc("TRN2", target_bir_lowering=False, debug=False)
    logits = nc.dram_tensor("logits", [n_tok, E], f32, kind="ExternalInput")
    # y_out row = [idx0..idx7 (u32 bit patterns), v0..v7]; DMAing the whole
    # tile keeps the store contiguous (one descriptor per partition) — the
    # host reads the idx half and ignores the value half. w is the softmax.
    y_out = nc.dram_tensor("y_out", [n_tok, 16], f32, kind="ExternalOutput")
    w_out = nc.dram_tensor("w_out", [n_tok, K], f32, kind="ExternalOutput")

    lg_v = logits.ap().rearrange("(p t) e -> p t e", p=P, t=tpp)
    y_v = y_out.ap().rearrange("(p t) k -> p t k", p=P, t=tpp)
    w_v = w_out.ap().rearrange("(p t) k -> p t k", p=P, t=tpp)

    with tile.TileContext(nc) as tc:
        with tc.tile_pool(name="io", bufs=5) as pool:

            def softmax_tail(tt, o, y, ex):
                """Softmax on the DVE (reduce + reciprocal + scale). The DVE
                custom op's dual-stream SBUF traffic starves the GPSIMD Q7s
                (~10x slowdown while it runs), so the softmax runs on the DVE
                queue itself; emitted one tile late, the inputs are always
                ready and these ~1.3us never stall the queue."""
                s = pool.tile([P, tt, 1], f32, tag="s")
                nc.vector.tensor_reduce(
                    s[:], ex[:], axis=mybir.AxisListType.X,
                    op=mybir.AluOpType.add,
                )
                r = pool.tile([P, tt, 1], f32, tag="r")
                nc.vector.reciprocal(r[:], s[:])
                w = pool.tile([P, tt, K], f32, tag="w")
                nc.vector.tensor_tensor(
                    w[:], ex[:], r[:].broadcast_to([P, tt, K]),
                    op=mybir.AluOpType.mult,
                )
                # output DMAs issue from the Pool queue: the serial Sync queue
                # carries ONLY input loads, so the next tile's x transfer is
                # never queued behind output DMAs waiting on the softmax.
                nc.gpsimd.dma_start(y_v[:, o:o + tt, :], y[:])
                nc.gpsimd.dma_start(w_v[:, o:o + tt, :], w[:])

            prev = None
            for o, tt in zip(offs, sizes):
                # one extra trailing element feeds the LT_8 termination issue
                x = pool.tile([P, tt * E + 1], f32, tag="x")
                x3 = x[:, 0:tt * E].rearrange("p (t e) -> p t e", t=tt, e=E)
                nc.sync.dma_start(x3, lg_v[:, o:o + tt, :])
                y = pool.tile([P, tt, 16], f32, tag="y")
                nc.vector._custom_dve(op, out=y[:], in0=x[:], in1=x3,
                                      s0=0.0, s1=0.0)
                ex = pool.tile([P, tt, K], f32, tag="ex")
                nc.scalar.activation(
                    ex[:], y[:, :, 8:16], mybir.ActivationFunctionType.Exp
                )
                if prev is not None:
                    softmax_tail(*prev)
                prev = (tt, o, y, ex)
            softmax_tail(*prev)
    nc.compile()
    return nc


def _get_nc():
    if "nc" not in _CACHE:
        _CACHE["nc"] = _build()
    return _CACHE["nc"]


def kernel(logits, top_k):
    logits = np.asarray(logits, dtype=np.float32)
    k = int(np.asarray(top_k))
    assert k == K, f"kernel hardcodes top_k={K}, got {k}"
    assert logits.shape == (N_TOKENS, E), logits.shape

    from concourse.bass_utils import run_bass_kernel_spmd

    nc = _get_nc()
    chunks = logits.reshape(NCORES, TPC, E)
    in_maps = [{"logits": np.ascontiguousarray(chunks[c])} for c in range(NCORES)]
    # The tunneled devices occasionally fail a run with a transient
    # NRT_EXEC_UNIT_UNRECOVERABLE error; a straight retry recovers.
    last_err = None
    for _attempt in range(3):
        try:
            res = run_bass_kernel_spmd(nc, in_maps, list(range(NCORES)))
            break
        except Exception as e:  # noqa: BLE001 - retry transient device faults
            last_err = e
            import time as _time

            _time.sleep(5.0)
    else:
        raise last_err

    # Row r of each per-core output is token r of that core's slice, so a
    # plain concat along the token axis reassembles the full outputs.
    y = np.concatenate([r["y_out"] for r in res.results], axis=0)
    w = np.concatenate([r["w_out"] for r in res.results], axis=0)
    idx = np.ascontiguousarray(y[:, 0:K]).view(np.uint32).astype(np.int64)
    return idx, w.astype(np.float32)


# revision 14
# speedup vs baseline: 1.3808x; 1.0156x over previous
"""Trainium2 Bass kernel: MoE top-k router (top-8 of 64 experts + softmax).

Contract: kernel(logits, top_k) takes the FULL inputs (logits [1048576, 64]
f32, top_k == 8) and returns (topk_idx int64 [N, 8], topk_w f32 [N, 8]),
matching jax.lax.top_k + jax.nn.softmax semantics (stable descending order,
ties broken toward the smaller index).

Sharding: data-parallel over tokens across 8 NeuronCores (one SPMD program,
per-core slices fed via run_bass_kernel_spmd). Per core, tokens are laid out
partition-major — partition p owns tokens [p*1024, (p+1)*1024).

The top-8 selection runs as ONE hand-written custom DVE instruction per
[128, T, 64] tile (vs 3 stock match-unit instructions per 128 tokens): a
MAX8-style swap-flop MIN-cascade streams each token's 64 logits from SRC_0
(slice k's swap flop retains the (k+1)-th largest), then a FIND_INDEX8-style
IS_EQ match pass re-streams the same 64 SBUF words through the second read
port (SRC_1) latching each slice's match position, then 8 match indices and
8 values drain and the uOp chain loops to the next token. ~146 DVE cycles
per 128-token group instead of ~310. Indices drain as raw u32 bit patterns
into the f32 output tile; the host reinterprets them (match HW handles
duplicate values in jax tie order — verified on planted duplicates).

Softmax of the 8 selected values stays off the DVE: exp on ScalarE, the
denominator tree-sum and final scale on GPSIMD, reciprocal via the 1-cycle
DVE approx seed + Newton-Raphson refinement on GPSIMD (exact DVE reciprocal
on the final small tile to shorten the serial tail).
"""

import sys

if "/opt/trn_rl_repo" not in sys.path:
    sys.path.insert(0, "/opt/trn_rl_repo")

from dataclasses import dataclass

import numpy as np

N_TOKENS = 1048576
E = 64             # experts
K = 8              # top-k
NCORES = 8
P = 128            # SBUF partitions
TPC = N_TOKENS // NCORES   # tokens per core = 131072
TPP = TPC // P             # tokens per partition = 1024
T = 64                     # tokens per partition per full tile

_CACHE = {}


# --------------------------------------------------------------------------
# Custom fused top-8 DVE op (values + match indices in one instruction).
#
# uOp chain (intra-spec indices; tok_len = 64):
#   0: RAMP0 entry (elem 0 seeds stage-0 swap)   SRC_DONE->IDLE, CNT1->2
#   1: RAMP0 loop  (same config; loop target)    SRC_DONE->IDLE, CNT1->2
#   2..8: RAMP1..7 (MIN cascade j<k + seed k)    CNT1->next
#   9: STEADY_A    (8-stage MIN cascade)         CNT56->10
#  10: CLEAR       (clear_match bubble)          CNT1->11
#  11: STEADY_B    (IS_EQ(stream, swap), latch)  CNT64->12   [reads SRC_1]
#  12: SPACER      (pipeline flush bubble)       CNT1->13
#  13: IDX_DRAIN   (8x OutSel.MATCH_INDEX)       CNT8->14
#  14..21: VDRAIN s0..s7 (descending values)     CNT1->next; last->1 (loop)
# --------------------------------------------------------------------------

def _build_topk_uops(tok_len=64):
    from concourse.dve_uop import (
        AluInp, AluOp, InpSel, OutPath, OutSel, Trigger, UopConfig, ENABLE,
    )

    def ramp(k):
        u = UopConfig()
        u.enable_input(InpSel.SRC_0, 0)
        u.require_inp0 = ENABLE
        u.repeat_count = 1
        for j in range(k):
            u.datapath_config[j].enable_alu(
                AluOp.MIN, AluInp.CURR_SWAP_OUT, AluInp.PREV_ALU_OUT
            )
            u.datapath_config[j].swap_enable = ENABLE
        u.datapath_config[k].enable_alu(
            AluOp.BYPASS, AluInp.PREV_ALU_OUT, AluInp.PREV_ALU_OUT
        )
        u.datapath_config[k].swap_enable = ENABLE
        return u

    uops = []
    # Termination: src0's AP carries ONE extra element past the last token, so
    # ramp0 always has data to issue (it never stalls on requires_src0) and
    # the level-evaluated SRC_TENSOR_LT_8 fires on that issue cycle -> IDLE.
    # (Waiting stalled on SRC_TENSOR_DONE after the stream drained misses the
    # done event and leaves the uOp FSM stalled past instruction retirement,
    # wedging the engine for the next NEFF execution.)
    for _ in (0, 1):  # 0: entry, 1: loop re-entry (next_uop 0 means IDLE)
        u = ramp(0)
        u.trigger = (Trigger.SRC_TENSOR_LT_8, Trigger.COUNT, Trigger.NONE)
        u.next_uop = (0, 2, 0)
        uops.append(u)
    for k in range(1, 8):
        u = ramp(k)
        u.trigger = (Trigger.COUNT, Trigger.NONE, Trigger.NONE)
        u.next_uop = (k + 2, 0, 0)
        uops.append(u)

    u = UopConfig()  # 9: steady_A
    u.enable_input(InpSel.SRC_0, 0)
    u.require_inp0 = ENABLE
    u.repeat_count = tok_len - 8
    for j in range(8):
        u.datapath_config[j].enable_alu(
            AluOp.MIN, AluInp.PREV_ALU_OUT, AluInp.CURR_SWAP_OUT
        )
        u.datapath_config[j].swap_enable = ENABLE
    u.trigger = (Trigger.COUNT, Trigger.NONE, Trigger.NONE)
    u.next_uop = (10, 0, 0)
    uops.append(u)

    u = UopConfig()  # 10: clear_match bubble (clear_match on the compare uop
    # itself re-clears every cycle and loses all but the last element's match)
    u.repeat_count = 1
    u.clear_match = ENABLE
    u.trigger = (Trigger.COUNT, Trigger.NONE, Trigger.NONE)
    u.next_uop = (11, 0, 0)
    uops.append(u)

    u = UopConfig()  # 11: steady_B — match pass over the second read port
    u.enable_input(InpSel.SRC_1, 1)  # lane 1 -> delay chain 0
    u.require_inp1 = ENABLE
    u.repeat_count = tok_len
    u.valid_match = ENABLE
    for j in range(8):
        u.datapath_config[j].enable_alu(
            AluOp.IS_EQ, AluInp.PREV_DELAY_0, AluInp.CURR_SWAP_OUT
        )
        u.datapath_config[j].pass_through_delay(0)
    u.trigger = (Trigger.COUNT, Trigger.NONE, Trigger.NONE)
    u.next_uop = (12, 0, 0)
    uops.append(u)

    u = UopConfig()  # 12: spacer (pipeline flush before latch readout)
    u.repeat_count = 1
    u.trigger = (Trigger.COUNT, Trigger.NONE, Trigger.NONE)
    u.next_uop = (13, 0, 0)
    uops.append(u)

    u = UopConfig()  # 13: idx drain
    u.repeat_count = 8
    u.enable_output(OutSel.MATCH_INDEX, OutPath.WR0_LO)
    u.trigger = (Trigger.COUNT, Trigger.NONE, Trigger.NONE)
    u.next_uop = (14, 0, 0)
    uops.append(u)

    for k in range(8):  # 14..21: value drains, slice 0 (largest) first
        u = UopConfig()
        u.repeat_count = 1
        u.datapath_config[k].enable_alu(
            AluOp.BYPASS, AluInp.CURR_SWAP_OUT, AluInp.CURR_SWAP_OUT
        )
        for j in range(k + 1, 8):
            u.datapath_config[j].pass_through_alu()
        u.enable_output(OutSel.ALU_OUT, OutPath.WR0_LO)
        u.trigger = (Trigger.COUNT, Trigger.NONE, Trigger.NONE)
        u.next_uop = (15 + k if k < 7 else 1, 0, 0)
        uops.append(u)
    return uops


def _get_topk_op():
    if "op" in _CACHE:
        return _CACHE["op"]
    from concourse.dve_ops import (
        DveOp, OPS, CUSTOM_DVE_SPECS, _SUB_OPCODE_FOR_NAME, get_dve_sub_opcode,
    )
    from concourse.dve_spec import Spec, Src0, Src1
    from concourse.dve_uop import DveOpSpec

    uops = _build_topk_uops(E)
    # op name carries the uop-bytes hash: a uop edit changes the BIR and so
    # the NEFF cache key, preventing stale-table reuse.
    tag = DveOpSpec(name="probe", opcode=1, uops=uops, rd1_en=True).sha("v3")[:8]
    name = f"TOPK8_{tag}"

    @dataclass(frozen=True)
    class RawDveOp(DveOp):
        raw_uops: tuple = ()

        def compile(self, ver):
            assert ver == "v3", f"hand-written for TRN2/v3 only, got {ver}"
            return DveOpSpec(
                name=self.name,
                opcode=get_dve_sub_opcode(self.name),
                uops=list(self.raw_uops),
                rd1_en=True,
            )

    def _ref(in0, in1, s0, s1, imm2):
        # CoreSim-only; the HW path never calls this.
        p = in0.shape[0]
        x = in0.reshape(p, -1, E)
        t = x.shape[1]
        out = np.zeros((p, t * 16), dtype=np.float32)
        order = np.argsort(-x, axis=-1, kind="stable")[..., :8]
        vals = np.take_along_axis(x, order, axis=-1)
        out.reshape(p, t, 16)[:, :, 0:8] = order.astype(np.uint32).view(np.float32)
        out.reshape(p, t, 16)[:, :, 8:16] = vals
        return out

    op = RawDveOp(
        name=name,
        spec=Spec(body=Src0 + Src1, reference=_ref),
        subdim=False,
        uops_sha={},
        raw_uops=tuple(uops),
    )
    if name not in _SUB_OPCODE_FOR_NAME:
        row = max(_SUB_OPCODE_FOR_NAME.values()) + 1
        assert row < 0x20, f"row {row} overflows the 5-bit byte-36 field"
        OPS.append(op)
        CUSTOM_DVE_SPECS[op.name] = op.spec
        _SUB_OPCODE_FOR_NAME[op.name] = row
    _CACHE["op"] = op
    return op


def _build(tpp=TPP, t_tile=T):
    import concourse.bacc as bacc
    import concourse.mybir as mybir
    import concourse.tile as tile

    f32 = mybir.dt.float32
    op = _get_topk_op()

    n_tok = P * tpp
    # small first tiles (DVE starts after ~256KB of DMA instead of 2MB) and a
    # small last tile (short softmax/store tail after the final DVE instr)
    if tpp == 1024 and t_tile == 64:
        sizes = [8, 56] + [64] * 14 + [48, 16]
    else:
        sizes = [t_tile] * (tpp // t_tile)
    assert sum(sizes) == tpp
    offs = [sum(sizes[:j]) for j in range(len(sizes))]

    nc = bacc.Bacc("TRN2", target_bir_lowering=False, debug=False)
    logits = nc.dram_tensor("logits", [n_tok, E], f32, kind="ExternalInput")
    # y_out row = [idx0..idx7 (u32 bit patterns), v0..v7]; DMAing the whole
    # tile keeps the store contiguous (one descriptor per partition) — the
    # host reads the idx half and ignores the value half. w is the softmax.
    y_out = nc.dram_tensor("y_out", [n_tok, 16], f32, kind="ExternalOutput")
    w_out = nc.dram_tensor("w_out", [n_tok, K], f32, kind="ExternalOutput")

    lg_v = logits.ap().rearrange("(p t) e -> p t e", p=P, t=tpp)
    y_v = y_out.ap().rearrange("(p t) k -> p t k", p=P, t=tpp)
    w_v = w_out.ap().rearrange("(p t) k -> p t k", p=P, t=tpp)

    with tile.TileContext(nc) as tc:
        with tc.tile_pool(name="io", bufs=5) as pool:

            def softmax_tail(tt, o, y, ex):
                """Softmax on the DVE (reduce + reciprocal + scale). The DVE
                custom op's dual-stream SBUF traffic starves the GPSIMD Q7s
                (~10x slowdown while it runs), so the softmax runs on the DVE
                queue itself; emitted one tile late, the inputs are always
                ready and these ~1.3us never stall the queue."""
                s = pool.tile([P, tt, 1], f32, tag="s")
                nc.vector.tensor_reduce(
                    s[:], ex[:], axis=mybir.AxisListType.X,
                    op=mybir.AluOpType.add,
                )
                r = pool.tile([P, tt, 1], f32, tag="r")
                # ~51-ULP single-pass approx (exact divide iterates 8 cycles
                # per element); 4e-6 relative on w, far inside the tolerance
                nc.vector.reciprocal_approx_fast(r[:], s[:])
                w = pool.tile([P, tt, K], f32, tag="w")
                nc.vector.tensor_tensor(
                    w[:], ex[:], r[:].broadcast_to([P, tt, K]),
                    op=mybir.AluOpType.mult,
                )
                # output DMAs issue from the Pool queue: the serial Sync queue
                # carries ONLY input loads, so the next tile's x transfer is
                # never queued behind output DMAs waiting on the softmax.
                nc.gpsimd.dma_start(y_v[:, o:o + tt, :], y[:])
                nc.gpsimd.dma_start(w_v[:, o:o + tt, :], w[:])

            prev = None
            for o, tt in zip(offs, sizes):
                # one extra trailing element feeds the LT_8 termination issue
                x = pool.tile([P, tt * E + 1], f32, tag="x")
                x3 = x[:, 0:tt * E].rearrange("p (t e) -> p t e", t=tt, e=E)
                nc.sync.dma_start(x3, lg_v[:, o:o + tt, :])
                y = pool.tile([P, tt, 16], f32, tag="y")
                nc.vector._custom_dve(op, out=y[:], in0=x[:], in1=x3,
                                      s0=0.0, s1=0.0)
                ex = pool.tile([P, tt, K], f32, tag="ex")
                nc.scalar.activation(
                    ex[:], y[:, :, 8:16], mybir.ActivationFunctionType.Exp
                )
                if prev is not None:
                    softmax_tail(*prev)
                prev = (tt, o, y, ex)
            softmax_tail(*prev)
    nc.compile()
    return nc


def _get_nc():
    if "nc" not in _CACHE:
        _CACHE["nc"] = _build()
    return _CACHE["nc"]


def kernel(logits, top_k):
    logits = np.asarray(logits, dtype=np.float32)
    k = int(np.asarray(top_k))
    assert k == K, f"kernel hardcodes top_k={K}, got {k}"
    assert logits.shape == (N_TOKENS, E), logits.shape

    from concourse.bass_utils import run_bass_kernel_spmd

    nc = _get_nc()
    chunks = logits.reshape(NCORES, TPC, E)
    in_maps = [{"logits": np.ascontiguousarray(chunks[c])} for c in range(NCORES)]
    # The tunneled devices occasionally fail a run with a transient
    # NRT_EXEC_UNIT_UNRECOVERABLE error; a straight retry recovers.
    last_err = None
    for _attempt in range(3):
        try:
            res = run_bass_kernel_spmd(nc, in_maps, list(range(NCORES)))
            break
        except Exception as e:  # noqa: BLE001 - retry transient device faults
            last_err = e
            import time as _time

            _time.sleep(5.0)
    else:
        raise last_err

    # Row r of each per-core output is token r of that core's slice, so a
    # plain concat along the token axis reassembles the full outputs.
    y = np.concatenate([r["y_out"] for r in res.results], axis=0)
    w = np.concatenate([r["w_out"] for r in res.results], axis=0)
    idx = np.ascontiguousarray(y[:, 0:K]).view(np.uint32).astype(np.int64)
    return idx, w.astype(np.float32)


# revision 19
# speedup vs baseline: 1.4496x; 1.0498x over previous
"""Trainium2 Bass kernel: MoE top-k router (top-8 of 64 experts + softmax).

Contract: kernel(logits, top_k) takes the FULL inputs (logits [1048576, 64]
f32, top_k == 8) and returns (topk_idx int64 [N, 8], topk_w f32 [N, 8]),
matching jax.lax.top_k + jax.nn.softmax semantics (stable descending order,
ties broken toward the smaller index).

Sharding: data-parallel over tokens across 8 NeuronCores (one SPMD program,
per-core slices fed via run_bass_kernel_spmd). Per core, tokens are laid out
partition-major — partition p owns tokens [p*1024, (p+1)*1024).

The top-8 selection runs as ONE hand-written custom DVE instruction per
[128, T, 64] tile (vs 3 stock match-unit instructions per 128 tokens): a
MAX8-style swap-flop MIN-cascade streams each token's 64 logits from SRC_0
(slice k's swap flop retains the (k+1)-th largest), then a FIND_INDEX8-style
IS_EQ match pass re-streams the same 64 SBUF words through the second read
port (SRC_1) latching each slice's match position, then 8 match indices and
8 values drain and the uOp chain loops to the next token. ~146 DVE cycles
per 128-token group instead of ~310. Indices drain as raw u32 bit patterns
into the f32 output tile; the host reinterprets them (match HW handles
duplicate values in jax tie order — verified on planted duplicates).

Softmax of the 8 selected values stays off the DVE: exp on ScalarE, the
denominator tree-sum and final scale on GPSIMD, reciprocal via the 1-cycle
DVE approx seed + Newton-Raphson refinement on GPSIMD (exact DVE reciprocal
on the final small tile to shorten the serial tail).
"""

import sys

if "/opt/trn_rl_repo" not in sys.path:
    sys.path.insert(0, "/opt/trn_rl_repo")

from dataclasses import dataclass

import numpy as np

N_TOKENS = 1048576
E = 64             # experts
K = 8              # top-k
NCORES = 8
P = 128            # SBUF partitions
TPC = N_TOKENS // NCORES   # tokens per core = 131072
TPP = TPC // P             # tokens per partition = 1024
T = 64                     # tokens per partition per full tile

_CACHE = {}


# --------------------------------------------------------------------------
# Custom fused top-8 DVE op (values + match indices in one instruction).
#
# uOp chain (intra-spec indices; tok_len = 64):
#   0: RAMP0 entry (elem 0 seeds stage-0 swap)   SRC_DONE->IDLE, CNT1->2
#   1: RAMP0 loop  (same config; loop target)    SRC_DONE->IDLE, CNT1->2
#   2..8: RAMP1..7 (MIN cascade j<k + seed k)    CNT1->next
#   9: STEADY_A    (8-stage MIN cascade)         CNT56->10
#  10: CLEAR       (clear_match bubble)          CNT1->11
#  11: STEADY_B    (IS_EQ(stream, swap), latch)  CNT64->12   [reads SRC_1]
#  12: SPACER      (pipeline flush bubble)       CNT1->13
#  13: IDX_DRAIN   (8x OutSel.MATCH_INDEX)       CNT8->14
#  14..21: VDRAIN s0..s7 (descending values)     CNT1->next; last->1 (loop)
# --------------------------------------------------------------------------

def _build_topk_uops(tok_len=64):
    from concourse.dve_uop import (
        AluInp, AluOp, InpSel, OutPath, OutSel, Trigger, UopConfig, ENABLE,
    )

    def ramp(k):
        """Element k of a token: MIN-cascade through stages < k, then a
        swap-EXCHANGE at stage k: BYPASS(a=CURR_SWAP_OUT, b=chain elem) emits
        the PREVIOUS token's (k+1)-th largest (alu_out = a) while latching the
        new seed (swap <- b), so the 8 value drains ride the next token's ramp
        for free. Stages > k forward the emitted value to the write port."""
        u = UopConfig()
        u.enable_input(InpSel.SRC_0, 0)
        u.require_inp0 = ENABLE
        u.repeat_count = 1
        for j in range(k):
            u.datapath_config[j].enable_alu(
                AluOp.MIN, AluInp.CURR_SWAP_OUT, AluInp.PREV_ALU_OUT
            )
            u.datapath_config[j].swap_enable = ENABLE
        u.datapath_config[k].enable_alu(
            AluOp.BYPASS, AluInp.CURR_SWAP_OUT, AluInp.PREV_ALU_OUT
        )
        u.datapath_config[k].swap_enable = ENABLE
        for j in range(k + 1, 8):
            u.datapath_config[j].pass_through_alu()
        u.enable_output(OutSel.ALU_OUT, OutPath.WR0_LO)
        return u

    uops = []
    # Termination: src0's AP carries ONE extra element past the last token, so
    # ramp0 always has data to issue (it never stalls on requires_src0) and
    # the level-evaluated SRC_TENSOR_LT_8 fires on that issue cycle -> IDLE.
    # (Waiting stalled on SRC_TENSOR_DONE after the stream drained misses the
    # done event and leaves the uOp FSM stalled past instruction retirement,
    # wedging the engine for the next NEFF execution.)
    for _ in (0, 1):  # 0: entry, 1: loop re-entry (next_uop 0 means IDLE)
        u = ramp(0)
        # exit to the epilogue (drain the last token's remaining 7 values)
        u.trigger = (Trigger.SRC_TENSOR_LT_8, Trigger.COUNT, Trigger.NONE)
        u.next_uop = (14, 2, 0)
        uops.append(u)
    for k in range(1, 8):
        u = ramp(k)
        u.trigger = (Trigger.COUNT, Trigger.NONE, Trigger.NONE)
        u.next_uop = (k + 2, 0, 0)
        uops.append(u)

    u = UopConfig()  # 9: steady_A
    u.enable_input(InpSel.SRC_0, 0)
    u.require_inp0 = ENABLE
    u.repeat_count = tok_len - 8
    for j in range(8):
        u.datapath_config[j].enable_alu(
            AluOp.MIN, AluInp.PREV_ALU_OUT, AluInp.CURR_SWAP_OUT
        )
        u.datapath_config[j].swap_enable = ENABLE
    u.trigger = (Trigger.COUNT, Trigger.NONE, Trigger.NONE)
    u.next_uop = (10, 0, 0)
    uops.append(u)

    u = UopConfig()  # 10: clear_match bubble (clear_match on the compare uop
    # itself re-clears every cycle and loses all but the last element's match)
    u.repeat_count = 1
    u.clear_match = ENABLE
    u.trigger = (Trigger.COUNT, Trigger.NONE, Trigger.NONE)
    u.next_uop = (11, 0, 0)
    uops.append(u)

    u = UopConfig()  # 11: steady_B — match pass over the second read port
    u.enable_input(InpSel.SRC_1, 1)  # lane 1 -> delay chain 0
    u.require_inp1 = ENABLE
    u.repeat_count = tok_len
    u.valid_match = ENABLE
    for j in range(8):
        u.datapath_config[j].enable_alu(
            AluOp.IS_EQ, AluInp.PREV_DELAY_0, AluInp.CURR_SWAP_OUT
        )
        u.datapath_config[j].pass_through_delay(0)
    u.trigger = (Trigger.COUNT, Trigger.NONE, Trigger.NONE)
    u.next_uop = (12, 0, 0)
    uops.append(u)

    u = UopConfig()  # 12: spacer (pipeline flush before latch readout)
    u.repeat_count = 1
    u.trigger = (Trigger.COUNT, Trigger.NONE, Trigger.NONE)
    u.next_uop = (13, 0, 0)
    uops.append(u)

    u = UopConfig()  # 13: idx drain, then straight to the next token's ramp
    u.repeat_count = 8
    u.enable_output(OutSel.MATCH_INDEX, OutPath.WR0_LO)
    u.trigger = (Trigger.COUNT, Trigger.NONE, Trigger.NONE)
    u.next_uop = (1, 0, 0)
    uops.append(u)

    for k in range(1, 8):  # 14..20: epilogue value drains s1..s7 (last token;
        # its s0 was emitted by the ramp0 issue that took the LT_8 exit)
        u = UopConfig()
        u.repeat_count = 1
        u.datapath_config[k].enable_alu(
            AluOp.BYPASS, AluInp.CURR_SWAP_OUT, AluInp.CURR_SWAP_OUT
        )
        for j in range(k + 1, 8):
            u.datapath_config[j].pass_through_alu()
        u.enable_output(OutSel.ALU_OUT, OutPath.WR0_LO)
        u.trigger = (Trigger.COUNT, Trigger.NONE, Trigger.NONE)
        u.next_uop = (14 + k if k < 7 else 0, 0, 0)
        uops.append(u)
    return uops


def _get_topk_op():
    if "op" in _CACHE:
        return _CACHE["op"]
    from concourse.dve_ops import (
        DveOp, OPS, CUSTOM_DVE_SPECS, _SUB_OPCODE_FOR_NAME, get_dve_sub_opcode,
    )
    from concourse.dve_spec import Spec, Src0, Src1
    from concourse.dve_uop import DveOpSpec

    uops = _build_topk_uops(E)
    # op name carries the uop-bytes hash: a uop edit changes the BIR and so
    # the NEFF cache key, preventing stale-table reuse.
    tag = DveOpSpec(name="probe", opcode=1, uops=uops, rd1_en=True).sha("v3")[:8]
    name = f"TOPK8_{tag}"

    @dataclass(frozen=True)
    class RawDveOp(DveOp):
        raw_uops: tuple = ()

        def compile(self, ver):
            assert ver == "v3", f"hand-written for TRN2/v3 only, got {ver}"
            return DveOpSpec(
                name=self.name,
                opcode=get_dve_sub_opcode(self.name),
                uops=list(self.raw_uops),
                rd1_en=True,
            )

    def _ref(in0, in1, s0, s1, imm2):
        # CoreSim-only; the HW path never calls this.
        p = in0.shape[0]
        x = in0.reshape(p, -1, E)
        t = x.shape[1]
        out = np.zeros((p, t * 16), dtype=np.float32)
        order = np.argsort(-x, axis=-1, kind="stable")[..., :8]
        vals = np.take_along_axis(x, order, axis=-1)
        out.reshape(p, t, 16)[:, :, 0:8] = order.astype(np.uint32).view(np.float32)
        out.reshape(p, t, 16)[:, :, 8:16] = vals
        return out

    op = RawDveOp(
        name=name,
        spec=Spec(body=Src0 + Src1, reference=_ref),
        subdim=False,
        uops_sha={},
        raw_uops=tuple(uops),
    )
    if name not in _SUB_OPCODE_FOR_NAME:
        row = max(_SUB_OPCODE_FOR_NAME.values()) + 1
        assert row < 0x20, f"row {row} overflows the 5-bit byte-36 field"
        OPS.append(op)
        CUSTOM_DVE_SPECS[op.name] = op.spec
        _SUB_OPCODE_FOR_NAME[op.name] = row
    _CACHE["op"] = op
    return op


def _build(tpp=TPP, t_tile=T):
    import concourse.bacc as bacc
    import concourse.mybir as mybir
    import concourse.tile as tile

    f32 = mybir.dt.float32
    op = _get_topk_op()

    n_tok = P * tpp
    # small first tiles (DVE starts after ~256KB of DMA instead of 2MB) and a
    # small last tile (short softmax/store tail after the final DVE instr)
    if tpp == 1024 and t_tile == 64:
        sizes = [8, 56] + [64] * 14 + [48, 16]
    else:
        sizes = [t_tile] * (tpp // t_tile)
    assert sum(sizes) == tpp
    offs = [sum(sizes[:j]) for j in range(len(sizes))]

    nc = bacc.Bacc("TRN2", target_bir_lowering=False, debug=False)
    logits = nc.dram_tensor("logits", [n_tok, E], f32, kind="ExternalInput")
    # y_out row = [idx0..idx7 (u32 bit patterns), v0..v7]; DMAing the whole
    # tile keeps the store contiguous (one descriptor per partition) — the
    # host reads the idx half and ignores the value half. w is the softmax.
    y_out = nc.dram_tensor("y_out", [n_tok, 16], f32, kind="ExternalOutput")
    w_out = nc.dram_tensor("w_out", [n_tok, K], f32, kind="ExternalOutput")

    lg_v = logits.ap().rearrange("(p t) e -> p t e", p=P, t=tpp)
    y_v = y_out.ap().rearrange("(p t) k -> p t k", p=P, t=tpp)
    w_v = w_out.ap().rearrange("(p t) k -> p t k", p=P, t=tpp)

    with tile.TileContext(nc) as tc:
        with tc.tile_pool(name="io", bufs=5) as pool:

            def softmax_tail(tt, o, y, ex):
                """Softmax on the DVE (reduce + reciprocal + scale). The DVE
                custom op's dual-stream SBUF traffic starves the GPSIMD Q7s
                (~10x slowdown while it runs), so the softmax runs on the DVE
                queue itself; emitted one tile late, the inputs are always
                ready and these ~1.3us never stall the queue."""
                s = pool.tile([P, tt, 1], f32, tag="s")
                nc.vector.tensor_reduce(
                    s[:], ex[:], axis=mybir.AxisListType.X,
                    op=mybir.AluOpType.add,
                )
                r = pool.tile([P, tt, 1], f32, tag="r")
                # ~51-ULP single-pass approx (exact divide iterates 8 cycles
                # per element); 4e-6 relative on w, far inside the tolerance
                nc.vector.reciprocal_approx_fast(r[:], s[:])
                w = pool.tile([P, tt, K], f32, tag="w")
                nc.vector.tensor_tensor(
                    w[:], ex[:], r[:].broadcast_to([P, tt, K]),
                    op=mybir.AluOpType.mult,
                )
                # output DMAs issue from the Pool queue: the serial Sync queue
                # carries ONLY input loads, so the next tile's x transfer is
                # never queued behind output DMAs waiting on the softmax.
                nc.gpsimd.dma_start(y_v[:, o:o + tt, :], y[:])
                nc.gpsimd.dma_start(w_v[:, o:o + tt, :], w[:])

            prev = None
            for o, tt in zip(offs, sizes):
                # one extra trailing element feeds the LT_8 termination issue
                x = pool.tile([P, tt * E + 1], f32, tag="x")
                x3 = x[:, 0:tt * E].rearrange("p (t e) -> p t e", t=tt, e=E)
                nc.sync.dma_start(x3, lg_v[:, o:o + tt, :])
                # output stream: 8 garbage words (stale swap flops emitted by
                # the first token's ramp), then per token [idx x8, vals x8]
                yr = pool.tile([P, tt * 16 + 8], f32, tag="y")
                y = yr[:, 8:].rearrange("p (t k) -> p t k", t=tt, k=16)
                nc.vector._custom_dve(op, out=yr[:], in0=x[:], in1=x3,
                                      s0=0.0, s1=0.0)
                ex = pool.tile([P, tt, K], f32, tag="ex")
                nc.scalar.activation(
                    ex[:], y[:, :, 8:16], mybir.ActivationFunctionType.Exp
                )
                if prev is not None:
                    softmax_tail(*prev)
                prev = (tt, o, y, ex)
            softmax_tail(*prev)
    nc.compile()
    return nc


def _get_nc():
    if "nc" not in _CACHE:
        _CACHE["nc"] = _build()
    return _CACHE["nc"]


def kernel(logits, top_k):
    logits = np.asarray(logits, dtype=np.float32)
    k = int(np.asarray(top_k))
    assert k == K, f"kernel hardcodes top_k={K}, got {k}"
    assert logits.shape == (N_TOKENS, E), logits.shape

    from concourse.bass_utils import run_bass_kernel_spmd

    nc = _get_nc()
    chunks = logits.reshape(NCORES, TPC, E)
    in_maps = [{"logits": np.ascontiguousarray(chunks[c])} for c in range(NCORES)]
    # The tunneled devices occasionally fail a run with a transient
    # NRT_EXEC_UNIT_UNRECOVERABLE error; a straight retry recovers.
    last_err = None
    for _attempt in range(3):
        try:
            res = run_bass_kernel_spmd(nc, in_maps, list(range(NCORES)))
            break
        except Exception as e:  # noqa: BLE001 - retry transient device faults
            last_err = e
            import time as _time

            _time.sleep(5.0)
    else:
        raise last_err

    # Row r of each per-core output is token r of that core's slice, so a
    # plain concat along the token axis reassembles the full outputs.
    y = np.concatenate([r["y_out"] for r in res.results], axis=0)
    w = np.concatenate([r["w_out"] for r in res.results], axis=0)
    idx = np.ascontiguousarray(y[:, 0:K]).view(np.uint32).astype(np.int64)
    return idx, w.astype(np.float32)


# revision 24
# speedup vs baseline: 1.4746x; 1.0173x over previous
"""Trainium2 Bass kernel: MoE top-k router (top-8 of 64 experts + softmax).

Contract: kernel(logits, top_k) takes the FULL inputs (logits [1048576, 64]
f32, top_k == 8) and returns (topk_idx int64 [N, 8], topk_w f32 [N, 8]),
matching jax.lax.top_k + jax.nn.softmax semantics (stable descending order,
ties broken toward the smaller index).

Sharding: data-parallel over tokens across 8 NeuronCores (one SPMD program,
per-core slices fed via run_bass_kernel_spmd). Per core, tokens are laid out
partition-major — partition p owns tokens [p*1024, (p+1)*1024).

The top-8 selection runs as ONE hand-written custom DVE instruction per
[128, T, 64] tile (vs 3 stock match-unit instructions per 128 tokens): a
MAX8-style swap-flop MIN-cascade streams each token's 64 logits from SRC_0
(slice k's swap flop retains the (k+1)-th largest), then a FIND_INDEX8-style
IS_EQ match pass re-streams the same 64 SBUF words through the second read
port (SRC_1) latching each slice's match position, then 8 match indices and
8 values drain and the uOp chain loops to the next token. ~146 DVE cycles
per 128-token group instead of ~310. Indices drain as raw u32 bit patterns
into the f32 output tile; the host reinterprets them (match HW handles
duplicate values in jax tie order — verified on planted duplicates).

Softmax of the 8 selected values stays off the DVE: exp on ScalarE, the
denominator tree-sum and final scale on GPSIMD, reciprocal via the 1-cycle
DVE approx seed + Newton-Raphson refinement on GPSIMD (exact DVE reciprocal
on the final small tile to shorten the serial tail).
"""

import sys

if "/opt/trn_rl_repo" not in sys.path:
    sys.path.insert(0, "/opt/trn_rl_repo")

from dataclasses import dataclass

import numpy as np

N_TOKENS = 1048576
E = 64             # experts
K = 8              # top-k
NCORES = 8
P = 128            # SBUF partitions
TPC = N_TOKENS // NCORES   # tokens per core = 131072
TPP = TPC // P             # tokens per partition = 1024
T = 64                     # tokens per partition per full tile

_CACHE = {}


# --------------------------------------------------------------------------
# Custom fused top-8 DVE op (values + match indices in one instruction).
#
# uOp chain (intra-spec indices; tok_len = 64):
#   0: RAMP0 entry (elem 0 seeds stage-0 swap)   SRC_DONE->IDLE, CNT1->2
#   1: RAMP0 loop  (same config; loop target)    SRC_DONE->IDLE, CNT1->2
#   2..8: RAMP1..7 (MIN cascade j<k + seed k)    CNT1->next
#   9: STEADY_A    (8-stage MIN cascade)         CNT56->10
#  10: CLEAR       (clear_match bubble)          CNT1->11
#  11: STEADY_B    (IS_EQ(stream, swap), latch)  CNT64->12   [reads SRC_1]
#  12: SPACER      (pipeline flush bubble)       CNT1->13
#  13: IDX_DRAIN   (8x OutSel.MATCH_INDEX)       CNT8->14
#  14..21: VDRAIN s0..s7 (descending values)     CNT1->next; last->1 (loop)
# --------------------------------------------------------------------------

def _build_topk_uops(tok_len=64):
    from concourse.dve_uop import (
        AluInp, AluOp, InpSel, OutPath, OutSel, Trigger, UopConfig, ENABLE,
    )

    def ramp(k):
        """Element k of a token: MIN-cascade through stages < k, then a
        swap-EXCHANGE at stage k: BYPASS(a=CURR_SWAP_OUT, b=chain elem) emits
        the PREVIOUS token's (k+1)-th largest (alu_out = a) while latching the
        new seed (swap <- b), so the 8 value drains ride the next token's ramp
        for free. Stages > k forward the emitted value to the write port."""
        u = UopConfig()
        u.enable_input(InpSel.SRC_0, 0)
        u.require_inp0 = ENABLE
        u.repeat_count = 1
        for j in range(k):
            u.datapath_config[j].enable_alu(
                AluOp.MIN, AluInp.CURR_SWAP_OUT, AluInp.PREV_ALU_OUT
            )
            u.datapath_config[j].swap_enable = ENABLE
        u.datapath_config[k].enable_alu(
            AluOp.BYPASS, AluInp.CURR_SWAP_OUT, AluInp.PREV_ALU_OUT
        )
        u.datapath_config[k].swap_enable = ENABLE
        for j in range(k + 1, 8):
            u.datapath_config[j].pass_through_alu()
        u.enable_output(OutSel.ALU_OUT, OutPath.WR0_LO)
        return u

    uops = []
    # Termination: src0's AP carries ONE extra element past the last token, so
    # ramp0 always has data to issue (it never stalls on requires_src0) and
    # the level-evaluated SRC_TENSOR_LT_8 fires on that issue cycle -> IDLE.
    # (Waiting stalled on SRC_TENSOR_DONE after the stream drained misses the
    # done event and leaves the uOp FSM stalled past instruction retirement,
    # wedging the engine for the next NEFF execution.)
    for _ in (0, 1):  # 0: entry, 1: loop re-entry (next_uop 0 means IDLE)
        u = ramp(0)
        # clear_match rides here: fires at this uop's entry, one cycle after
        # the previous token's idx drain issued its last readout, and the
        # latches stay untouched until this token's own match pass
        u.clear_match = ENABLE
        # exit to the epilogue (drain the last token's remaining 7 values)
        u.trigger = (Trigger.SRC_TENSOR_LT_8, Trigger.COUNT, Trigger.NONE)
        u.next_uop = (13, 2, 0)
        uops.append(u)
    for k in range(1, 8):
        u = ramp(k)
        u.trigger = (Trigger.COUNT, Trigger.NONE, Trigger.NONE)
        u.next_uop = (k + 2, 0, 0)
        uops.append(u)

    u = UopConfig()  # 9: steady_A
    u.enable_input(InpSel.SRC_0, 0)
    u.require_inp0 = ENABLE
    u.repeat_count = tok_len - 8
    for j in range(8):
        u.datapath_config[j].enable_alu(
            AluOp.MIN, AluInp.PREV_ALU_OUT, AluInp.CURR_SWAP_OUT
        )
        u.datapath_config[j].swap_enable = ENABLE
    u.trigger = (Trigger.COUNT, Trigger.NONE, Trigger.NONE)
    u.next_uop = (10, 0, 0)
    uops.append(u)

    u = UopConfig()  # 10: steady_B — match pass over the second read port
    u.enable_input(InpSel.SRC_1, 1)  # lane 1 -> delay chain 0
    u.require_inp1 = ENABLE
    u.repeat_count = tok_len
    u.valid_match = ENABLE
    for j in range(8):
        u.datapath_config[j].enable_alu(
            AluOp.IS_EQ, AluInp.PREV_DELAY_0, AluInp.CURR_SWAP_OUT
        )
        u.datapath_config[j].pass_through_delay(0)
    u.trigger = (Trigger.COUNT, Trigger.NONE, Trigger.NONE)
    u.next_uop = (11, 0, 0)
    uops.append(u)

    u = UopConfig()  # 11: spacer (pipeline flush before latch readout)
    u.repeat_count = 1
    u.trigger = (Trigger.COUNT, Trigger.NONE, Trigger.NONE)
    u.next_uop = (12, 0, 0)
    uops.append(u)

    u = UopConfig()  # 12: idx drain, then straight to the next token's ramp
    u.repeat_count = 8
    u.enable_output(OutSel.MATCH_INDEX, OutPath.WR0_LO)
    u.trigger = (Trigger.COUNT, Trigger.NONE, Trigger.NONE)
    u.next_uop = (1, 0, 0)
    uops.append(u)

    for k in range(1, 8):  # 13..19: epilogue value drains s1..s7 (last token;
        # its s0 was emitted by the ramp0 issue that took the LT_8 exit)
        u = UopConfig()
        u.repeat_count = 1
        u.datapath_config[k].enable_alu(
            AluOp.BYPASS, AluInp.CURR_SWAP_OUT, AluInp.CURR_SWAP_OUT
        )
        for j in range(k + 1, 8):
            u.datapath_config[j].pass_through_alu()
        u.enable_output(OutSel.ALU_OUT, OutPath.WR0_LO)
        u.trigger = (Trigger.COUNT, Trigger.NONE, Trigger.NONE)
        u.next_uop = (13 + k if k < 7 else 0, 0, 0)
        uops.append(u)
    return uops


def _get_topk_op():
    if "op" in _CACHE:
        return _CACHE["op"]
    from concourse.dve_ops import (
        DveOp, OPS, CUSTOM_DVE_SPECS, _SUB_OPCODE_FOR_NAME, get_dve_sub_opcode,
    )
    from concourse.dve_spec import Spec, Src0, Src1
    from concourse.dve_uop import DveOpSpec

    uops = _build_topk_uops(E)
    # op name carries the uop-bytes hash: a uop edit changes the BIR and so
    # the NEFF cache key, preventing stale-table reuse.
    tag = DveOpSpec(name="probe", opcode=1, uops=uops, rd1_en=True).sha("v3")[:8]
    name = f"TOPK8_{tag}"

    @dataclass(frozen=True)
    class RawDveOp(DveOp):
        raw_uops: tuple = ()

        def compile(self, ver):
            assert ver == "v3", f"hand-written for TRN2/v3 only, got {ver}"
            return DveOpSpec(
                name=self.name,
                opcode=get_dve_sub_opcode(self.name),
                uops=list(self.raw_uops),
                rd1_en=True,
            )

    def _ref(in0, in1, s0, s1, imm2):
        # CoreSim-only; the HW path never calls this.
        p = in0.shape[0]
        x = in0.reshape(p, -1, E)
        t = x.shape[1]
        out = np.zeros((p, t * 16), dtype=np.float32)
        order = np.argsort(-x, axis=-1, kind="stable")[..., :8]
        vals = np.take_along_axis(x, order, axis=-1)
        out.reshape(p, t, 16)[:, :, 0:8] = order.astype(np.uint32).view(np.float32)
        out.reshape(p, t, 16)[:, :, 8:16] = vals
        return out

    op = RawDveOp(
        name=name,
        spec=Spec(body=Src0 + Src1, reference=_ref),
        subdim=False,
        uops_sha={},
        raw_uops=tuple(uops),
    )
    if name not in _SUB_OPCODE_FOR_NAME:
        row = max(_SUB_OPCODE_FOR_NAME.values()) + 1
        assert row < 0x20, f"row {row} overflows the 5-bit byte-36 field"
        OPS.append(op)
        CUSTOM_DVE_SPECS[op.name] = op.spec
        _SUB_OPCODE_FOR_NAME[op.name] = row
    _CACHE["op"] = op
    return op


def _build(tpp=TPP, t_tile=T):
    import concourse.bacc as bacc
    import concourse.mybir as mybir
    import concourse.tile as tile

    f32 = mybir.dt.float32
    op = _get_topk_op()

    n_tok = P * tpp
    # small first tiles (DVE starts after ~512KB of DMA instead of 3MB) and a
    # small last tile (short softmax/store tail after the final DVE instr);
    # 96-token tiles keep the match-index stream under the counter range
    # (96*136 < 16384) while amortizing per-tile semaphore/queue overhead
    if tpp == 1024:
        sizes = [16, 48] + [96] * 9 + [64, 32]
    else:
        sizes = [t_tile] * (tpp // t_tile)
    assert sum(sizes) == tpp
    offs = [sum(sizes[:j]) for j in range(len(sizes))]

    nc = bacc.Bacc("TRN2", target_bir_lowering=False, debug=False)
    logits = nc.dram_tensor("logits", [n_tok, E], f32, kind="ExternalInput")
    # y_out row = [idx0..idx7 (u32 bit patterns), v0..v7]; DMAing the whole
    # tile keeps the store contiguous (one descriptor per partition) — the
    # host reads the idx half and ignores the value half. w is the softmax.
    y_out = nc.dram_tensor("y_out", [n_tok, 16], f32, kind="ExternalOutput")
    w_out = nc.dram_tensor("w_out", [n_tok, K], f32, kind="ExternalOutput")

    lg_v = logits.ap().rearrange("(p t) e -> p t e", p=P, t=tpp)
    y_v = y_out.ap().rearrange("(p t) k -> p t k", p=P, t=tpp)
    w_v = w_out.ap().rearrange("(p t) k -> p t k", p=P, t=tpp)

    with tile.TileContext(nc) as tc:
        with tc.tile_pool(name="io", bufs=5) as pool:

            def softmax_tail(tt, o, y, ex):
                """Softmax on the DVE (reduce + reciprocal + scale). The DVE
                custom op's dual-stream SBUF traffic starves the GPSIMD Q7s
                (~10x slowdown while it runs), so the softmax runs on the DVE
                queue itself; emitted one tile late, the inputs are always
                ready and these ~1.3us never stall the queue."""
                s = pool.tile([P, tt, 1], f32, tag="s")
                nc.vector.tensor_reduce(
                    s[:], ex[:], axis=mybir.AxisListType.X,
                    op=mybir.AluOpType.add,
                )
                r = pool.tile([P, tt, 1], f32, tag="r")
                # ~51-ULP single-pass approx (exact divide iterates 8 cycles
                # per element); 4e-6 relative on w, far inside the tolerance
                nc.vector.reciprocal_approx_fast(r[:], s[:])
                w = pool.tile([P, tt, K], f32, tag="w")
                nc.vector.tensor_tensor(
                    w[:], ex[:], r[:].broadcast_to([P, tt, K]),
                    op=mybir.AluOpType.mult,
                )
                # output DMAs issue from the Pool queue: the serial Sync queue
                # carries ONLY input loads, so the next tile's x transfer is
                # never queued behind output DMAs waiting on the softmax.
                nc.gpsimd.dma_start(y_v[:, o:o + tt, :], y[:])
                nc.gpsimd.dma_start(w_v[:, o:o + tt, :], w[:])

            prev = None
            for o, tt in zip(offs, sizes):
                # one extra trailing element feeds the LT_8 termination issue
                x = pool.tile([P, tt * E + 1], f32, tag="x")
                x3 = x[:, 0:tt * E].rearrange("p (t e) -> p t e", t=tt, e=E)
                nc.sync.dma_start(x3, lg_v[:, o:o + tt, :])
                # output stream: 8 garbage words (stale swap flops emitted by
                # the first token's ramp), then per token [idx x8, vals x8]
                yr = pool.tile([P, tt * 16 + 8], f32, tag="y")
                y = yr[:, 8:].rearrange("p (t k) -> p t k", t=tt, k=16)
                nc.vector._custom_dve(op, out=yr[:], in0=x[:], in1=x3,
                                      s0=0.0, s1=0.0)
                ex = pool.tile([P, tt, K], f32, tag="ex")
                nc.scalar.activation(
                    ex[:], y[:, :, 8:16], mybir.ActivationFunctionType.Exp
                )
                if prev is not None:
                    softmax_tail(*prev)
                prev = (tt, o, y, ex)
            softmax_tail(*prev)
    nc.compile()
    return nc


def _get_nc():
    if "nc" not in _CACHE:
        _CACHE["nc"] = _build()
    return _CACHE["nc"]


def kernel(logits, top_k):
    logits = np.asarray(logits, dtype=np.float32)
    k = int(np.asarray(top_k))
    assert k == K, f"kernel hardcodes top_k={K}, got {k}"
    assert logits.shape == (N_TOKENS, E), logits.shape

    from concourse.bass_utils import run_bass_kernel_spmd

    nc = _get_nc()
    chunks = logits.reshape(NCORES, TPC, E)
    in_maps = [{"logits": np.ascontiguousarray(chunks[c])} for c in range(NCORES)]
    # The tunneled devices occasionally fail a run with a transient
    # NRT_EXEC_UNIT_UNRECOVERABLE error; a straight retry recovers.
    last_err = None
    for _attempt in range(3):
        try:
            res = run_bass_kernel_spmd(nc, in_maps, list(range(NCORES)))
            break
        except Exception as e:  # noqa: BLE001 - retry transient device faults
            last_err = e
            import time as _time

            _time.sleep(5.0)
    else:
        raise last_err

    # Row r of each per-core output is token r of that core's slice, so a
    # plain concat along the token axis reassembles the full outputs.
    y = np.concatenate([r["y_out"] for r in res.results], axis=0)
    w = np.concatenate([r["w_out"] for r in res.results], axis=0)
    idx = np.ascontiguousarray(y[:, 0:K]).view(np.uint32).astype(np.int64)
    return idx, w.astype(np.float32)


# revision 27
# speedup vs baseline: 1.4830x; 1.0057x over previous
"""Trainium2 Bass kernel: MoE top-k router (top-8 of 64 experts + softmax).

Contract: kernel(logits, top_k) takes the FULL inputs (logits [1048576, 64]
f32, top_k == 8) and returns (topk_idx int64 [N, 8], topk_w f32 [N, 8]),
matching jax.lax.top_k + jax.nn.softmax semantics (stable descending order,
ties broken toward the smaller index).

Sharding: data-parallel over tokens across 8 NeuronCores (one SPMD program,
per-core slices fed via run_bass_kernel_spmd). Per core, tokens are laid out
partition-major — partition p owns tokens [p*1024, (p+1)*1024).

The top-8 selection runs as ONE hand-written custom DVE instruction per
[128, T, 64] tile (vs 3 stock match-unit instructions per 128 tokens): a
MAX8-style swap-flop MIN-cascade streams each token's 64 logits from SRC_0
(slice k's swap flop retains the (k+1)-th largest), then a FIND_INDEX8-style
IS_EQ match pass re-streams the same 64 SBUF words through the second read
port (SRC_1) latching each slice's match position, then 8 match indices and
8 values drain and the uOp chain loops to the next token. ~146 DVE cycles
per 128-token group instead of ~310. Indices drain as raw u32 bit patterns
into the f32 output tile; the host reinterprets them (match HW handles
duplicate values in jax tie order — verified on planted duplicates).

Softmax of the 8 selected values stays off the DVE: exp on ScalarE, the
denominator tree-sum and final scale on GPSIMD, reciprocal via the 1-cycle
DVE approx seed + Newton-Raphson refinement on GPSIMD (exact DVE reciprocal
on the final small tile to shorten the serial tail).
"""

import sys

if "/opt/trn_rl_repo" not in sys.path:
    sys.path.insert(0, "/opt/trn_rl_repo")

from dataclasses import dataclass

import numpy as np

N_TOKENS = 1048576
E = 64             # experts
K = 8              # top-k
NCORES = 8
P = 128            # SBUF partitions
TPC = N_TOKENS // NCORES   # tokens per core = 131072
TPP = TPC // P             # tokens per partition = 1024
T = 64                     # tokens per partition per full tile

_CACHE = {}


# --------------------------------------------------------------------------
# Custom fused top-8 DVE op (values + match indices in one instruction).
#
# uOp chain (intra-spec indices; tok_len = 64):
#   0: RAMP0 entry (elem 0 seeds stage-0 swap)   SRC_DONE->IDLE, CNT1->2
#   1: RAMP0 loop  (same config; loop target)    SRC_DONE->IDLE, CNT1->2
#   2..8: RAMP1..7 (MIN cascade j<k + seed k)    CNT1->next
#   9: STEADY_A    (8-stage MIN cascade)         CNT56->10
#  10: CLEAR       (clear_match bubble)          CNT1->11
#  11: STEADY_B    (IS_EQ(stream, swap), latch)  CNT64->12   [reads SRC_1]
#  12: SPACER      (pipeline flush bubble)       CNT1->13
#  13: IDX_DRAIN   (8x OutSel.MATCH_INDEX)       CNT8->14
#  14..21: VDRAIN s0..s7 (descending values)     CNT1->next; last->1 (loop)
# --------------------------------------------------------------------------

def _build_topk_uops(tok_len=64):
    from concourse.dve_uop import (
        AluInp, AluOp, InpSel, OutPath, OutSel, Trigger, UopConfig, ENABLE,
    )

    def ramp(k):
        """Element k of a token: MIN-cascade through stages < k, then a
        swap-EXCHANGE at stage k: BYPASS(a=CURR_SWAP_OUT, b=chain elem) emits
        the PREVIOUS token's (k+1)-th largest (alu_out = a) while latching the
        new seed (swap <- b), so the 8 value drains ride the next token's ramp
        for free. Stages > k forward the emitted value to the write port."""
        u = UopConfig()
        u.enable_input(InpSel.SRC_0, 0)
        u.require_inp0 = ENABLE
        u.repeat_count = 1
        for j in range(k):
            u.datapath_config[j].enable_alu(
                AluOp.MIN, AluInp.CURR_SWAP_OUT, AluInp.PREV_ALU_OUT
            )
            u.datapath_config[j].swap_enable = ENABLE
        u.datapath_config[k].enable_alu(
            AluOp.BYPASS, AluInp.CURR_SWAP_OUT, AluInp.PREV_ALU_OUT
        )
        u.datapath_config[k].swap_enable = ENABLE
        for j in range(k + 1, 8):
            u.datapath_config[j].pass_through_alu()
        u.enable_output(OutSel.ALU_OUT, OutPath.WR0_LO)
        return u

    uops = []
    # Termination: src0's AP carries ONE extra element past the last token, so
    # ramp0 always has data to issue (it never stalls on requires_src0) and
    # the level-evaluated SRC_TENSOR_LT_8 fires on that issue cycle -> IDLE.
    # (Waiting stalled on SRC_TENSOR_DONE after the stream drained misses the
    # done event and leaves the uOp FSM stalled past instruction retirement,
    # wedging the engine for the next NEFF execution.)
    for _ in (0, 1):  # 0: entry, 1: loop re-entry (next_uop 0 means IDLE)
        u = ramp(0)
        # clear_match rides here: fires at this uop's entry, one cycle after
        # the previous token's idx drain issued its last readout, and the
        # latches stay untouched until this token's own match pass
        u.clear_match = ENABLE
        # exit to the epilogue (drain the last token's remaining 7 values)
        u.trigger = (Trigger.SRC_TENSOR_LT_8, Trigger.COUNT, Trigger.NONE)
        u.next_uop = (13, 2, 0)
        uops.append(u)
    for k in range(1, 8):
        u = ramp(k)
        u.trigger = (Trigger.COUNT, Trigger.NONE, Trigger.NONE)
        u.next_uop = (k + 2, 0, 0)
        uops.append(u)

    u = UopConfig()  # 9: steady_A
    u.enable_input(InpSel.SRC_0, 0)
    u.require_inp0 = ENABLE
    u.repeat_count = tok_len - 8
    for j in range(8):
        u.datapath_config[j].enable_alu(
            AluOp.MIN, AluInp.PREV_ALU_OUT, AluInp.CURR_SWAP_OUT
        )
        u.datapath_config[j].swap_enable = ENABLE
    u.trigger = (Trigger.COUNT, Trigger.NONE, Trigger.NONE)
    u.next_uop = (10, 0, 0)
    uops.append(u)

    u = UopConfig()  # 10: steady_B — match pass over the second read port
    u.enable_input(InpSel.SRC_1, 1)  # lane 1 -> delay chain 0
    u.require_inp1 = ENABLE
    u.repeat_count = tok_len
    u.valid_match = ENABLE
    for j in range(8):
        u.datapath_config[j].enable_alu(
            AluOp.IS_EQ, AluInp.PREV_DELAY_0, AluInp.CURR_SWAP_OUT
        )
        u.datapath_config[j].pass_through_delay(0)
    u.trigger = (Trigger.COUNT, Trigger.NONE, Trigger.NONE)
    u.next_uop = (11, 0, 0)
    uops.append(u)

    u = UopConfig()  # 11: spacer — one flush cycle so a match at the LAST
    # stream position settles its latch before the readout samples it
    # (dropping this loses idx[0] whenever the top-1 sits at position 63)
    u.repeat_count = 1
    u.trigger = (Trigger.COUNT, Trigger.NONE, Trigger.NONE)
    u.next_uop = (12, 0, 0)
    uops.append(u)

    u = UopConfig()  # 12: idx drain, then straight to the next token's ramp
    u.repeat_count = 8
    u.enable_output(OutSel.MATCH_INDEX, OutPath.WR0_LO)
    u.trigger = (Trigger.COUNT, Trigger.NONE, Trigger.NONE)
    u.next_uop = (1, 0, 0)
    uops.append(u)

    for k in range(1, 8):  # 13..19: epilogue value drains s1..s7 (last token;
        # its s0 was emitted by the ramp0 issue that took the LT_8 exit)
        u = UopConfig()
        u.repeat_count = 1
        u.datapath_config[k].enable_alu(
            AluOp.BYPASS, AluInp.CURR_SWAP_OUT, AluInp.CURR_SWAP_OUT
        )
        for j in range(k + 1, 8):
            u.datapath_config[j].pass_through_alu()
        u.enable_output(OutSel.ALU_OUT, OutPath.WR0_LO)
        u.trigger = (Trigger.COUNT, Trigger.NONE, Trigger.NONE)
        u.next_uop = (13 + k if k < 7 else 0, 0, 0)
        uops.append(u)
    return uops


def _get_topk_op():
    if "op" in _CACHE:
        return _CACHE["op"]
    from concourse.dve_ops import (
        DveOp, OPS, CUSTOM_DVE_SPECS, _SUB_OPCODE_FOR_NAME, get_dve_sub_opcode,
    )
    from concourse.dve_spec import Spec, Src0, Src1
    from concourse.dve_uop import DveOpSpec

    uops = _build_topk_uops(E)
    # op name carries the uop-bytes hash: a uop edit changes the BIR and so
    # the NEFF cache key, preventing stale-table reuse.
    tag = DveOpSpec(name="probe", opcode=1, uops=uops, rd1_en=True).sha("v3")[:8]
    name = f"TOPK8_{tag}"

    @dataclass(frozen=True)
    class RawDveOp(DveOp):
        raw_uops: tuple = ()

        def compile(self, ver):
            assert ver == "v3", f"hand-written for TRN2/v3 only, got {ver}"
            return DveOpSpec(
                name=self.name,
                opcode=get_dve_sub_opcode(self.name),
                uops=list(self.raw_uops),
                rd1_en=True,
            )

    def _ref(in0, in1, s0, s1, imm2):
        # CoreSim-only; the HW path never calls this.
        p = in0.shape[0]
        x = in0.reshape(p, -1, E)
        t = x.shape[1]
        out = np.zeros((p, t * 16), dtype=np.float32)
        order = np.argsort(-x, axis=-1, kind="stable")[..., :8]
        vals = np.take_along_axis(x, order, axis=-1)
        out.reshape(p, t, 16)[:, :, 0:8] = order.astype(np.uint32).view(np.float32)
        out.reshape(p, t, 16)[:, :, 8:16] = vals
        return out

    op = RawDveOp(
        name=name,
        spec=Spec(body=Src0 + Src1, reference=_ref),
        subdim=False,
        uops_sha={},
        raw_uops=tuple(uops),
    )
    if name not in _SUB_OPCODE_FOR_NAME:
        row = max(_SUB_OPCODE_FOR_NAME.values()) + 1
        assert row < 0x20, f"row {row} overflows the 5-bit byte-36 field"
        OPS.append(op)
        CUSTOM_DVE_SPECS[op.name] = op.spec
        _SUB_OPCODE_FOR_NAME[op.name] = row
    _CACHE["op"] = op
    return op


def _build(tpp=TPP, t_tile=T):
    import concourse.bacc as bacc
    import concourse.mybir as mybir
    import concourse.tile as tile

    f32 = mybir.dt.float32
    op = _get_topk_op()

    n_tok = P * tpp
    # small first tiles (DVE starts after ~512KB of DMA instead of 3MB) and a
    # small last tile (short softmax/store tail after the final DVE instr);
    # 96-token tiles keep the match-index stream under the counter range
    # (96*136 < 16384) while amortizing per-tile semaphore/queue overhead
    if tpp == 1024:
        sizes = [16, 48] + [96] * 9 + [64, 32]
    else:
        sizes = [t_tile] * (tpp // t_tile)
    assert sum(sizes) == tpp
    offs = [sum(sizes[:j]) for j in range(len(sizes))]

    nc = bacc.Bacc("TRN2", target_bir_lowering=False, debug=False)
    logits = nc.dram_tensor("logits", [n_tok, E], f32, kind="ExternalInput")
    # y_out row = [idx0..idx7 (u32 bit patterns), v0..v7]; DMAing the whole
    # tile keeps the store contiguous (one descriptor per partition) — the
    # host reads the idx half and ignores the value half. w is the softmax.
    y_out = nc.dram_tensor("y_out", [n_tok, 16], f32, kind="ExternalOutput")
    w_out = nc.dram_tensor("w_out", [n_tok, K], f32, kind="ExternalOutput")

    lg_v = logits.ap().rearrange("(p t) e -> p t e", p=P, t=tpp)
    y_v = y_out.ap().rearrange("(p t) k -> p t k", p=P, t=tpp)
    w_v = w_out.ap().rearrange("(p t) k -> p t k", p=P, t=tpp)

    with tile.TileContext(nc) as tc:
        with tc.tile_pool(name="io", bufs=5) as pool:

            def softmax_tail(tt, o, y, ex):
                """Softmax on the DVE (reduce + reciprocal + scale). The DVE
                custom op's dual-stream SBUF traffic starves the GPSIMD Q7s
                (~10x slowdown while it runs), so the softmax runs on the DVE
                queue itself; emitted one tile late, the inputs are always
                ready and these ~1.3us never stall the queue."""
                s = pool.tile([P, tt, 1], f32, tag="s")
                nc.vector.tensor_reduce(
                    s[:], ex[:], axis=mybir.AxisListType.X,
                    op=mybir.AluOpType.add,
                )
                r = pool.tile([P, tt, 1], f32, tag="r")
                # ~51-ULP single-pass approx (exact divide iterates 8 cycles
                # per element); 4e-6 relative on w, far inside the tolerance
                nc.vector.reciprocal_approx_fast(r[:], s[:])
                w = pool.tile([P, tt, K], f32, tag="w")
                nc.vector.tensor_tensor(
                    w[:], ex[:], r[:].broadcast_to([P, tt, K]),
                    op=mybir.AluOpType.mult,
                )
                # output DMAs issue from the Pool queue: the serial Sync queue
                # carries ONLY input loads, so the next tile's x transfer is
                # never queued behind output DMAs waiting on the softmax.
                nc.gpsimd.dma_start(y_v[:, o:o + tt, :], y[:])
                nc.gpsimd.dma_start(w_v[:, o:o + tt, :], w[:])

            prev = None
            for o, tt in zip(offs, sizes):
                # one extra trailing element feeds the LT_8 termination issue
                x = pool.tile([P, tt * E + 1], f32, tag="x")
                x3 = x[:, 0:tt * E].rearrange("p (t e) -> p t e", t=tt, e=E)
                nc.sync.dma_start(x3, lg_v[:, o:o + tt, :])
                # output stream: 8 garbage words (stale swap flops emitted by
                # the first token's ramp), then per token [idx x8, vals x8]
                yr = pool.tile([P, tt * 16 + 8], f32, tag="y")
                y = yr[:, 8:].rearrange("p (t k) -> p t k", t=tt, k=16)
                nc.vector._custom_dve(op, out=yr[:], in0=x[:], in1=x3,
                                      s0=0.0, s1=0.0)
                ex = pool.tile([P, tt, K], f32, tag="ex")
                nc.scalar.activation(
                    ex[:], y[:, :, 8:16], mybir.ActivationFunctionType.Exp
                )
                if prev is not None:
                    softmax_tail(*prev)
                prev = (tt, o, y, ex)
            softmax_tail(*prev)
    nc.compile()
    return nc


def _get_nc():
    if "nc" not in _CACHE:
        _CACHE["nc"] = _build()
    return _CACHE["nc"]


def kernel(logits, top_k):
    logits = np.asarray(logits, dtype=np.float32)
    k = int(np.asarray(top_k))
    assert k == K, f"kernel hardcodes top_k={K}, got {k}"
    assert logits.shape == (N_TOKENS, E), logits.shape

    from concourse.bass_utils import run_bass_kernel_spmd

    nc = _get_nc()
    chunks = logits.reshape(NCORES, TPC, E)
    in_maps = [{"logits": np.ascontiguousarray(chunks[c])} for c in range(NCORES)]
    # The tunneled devices occasionally fail a run with a transient
    # NRT_EXEC_UNIT_UNRECOVERABLE error; a straight retry recovers.
    last_err = None
    for _attempt in range(3):
        try:
            res = run_bass_kernel_spmd(nc, in_maps, list(range(NCORES)))
            break
        except Exception as e:  # noqa: BLE001 - retry transient device faults
            last_err = e
            import time as _time

            _time.sleep(5.0)
    else:
        raise last_err

    # Row r of each per-core output is token r of that core's slice, so a
    # plain concat along the token axis reassembles the full outputs.
    y = np.concatenate([r["y_out"] for r in res.results], axis=0)
    w = np.concatenate([r["w_out"] for r in res.results], axis=0)
    idx = np.ascontiguousarray(y[:, 0:K]).view(np.uint32).astype(np.int64)
    return idx, w.astype(np.float32)


# revision 28
# speedup vs baseline: 1.5064x; 1.0158x over previous
"""Trainium2 Bass kernel: MoE top-k router (top-8 of 64 experts + softmax).

Contract: kernel(logits, top_k) takes the FULL inputs (logits [1048576, 64]
f32, top_k == 8) and returns (topk_idx int64 [N, 8], topk_w f32 [N, 8]),
matching jax.lax.top_k + jax.nn.softmax semantics (stable descending order,
ties broken toward the smaller index).

Sharding: data-parallel over tokens across 8 NeuronCores (one SPMD program,
per-core slices fed via run_bass_kernel_spmd). Per core, tokens are laid out
partition-major — partition p owns tokens [p*1024, (p+1)*1024).

The top-8 selection runs as ONE hand-written custom DVE instruction per
[128, T, 64] tile (vs 3 stock match-unit instructions per 128 tokens): a
MAX8-style swap-flop MIN-cascade streams each token's 64 logits from SRC_0
(slice k's swap flop retains the (k+1)-th largest), then a FIND_INDEX8-style
IS_EQ match pass re-streams the same 64 SBUF words through the second read
port (SRC_1) latching each slice's match position, then 8 match indices and
8 values drain and the uOp chain loops to the next token. ~146 DVE cycles
per 128-token group instead of ~310. Indices drain as raw u32 bit patterns
into the f32 output tile; the host reinterprets them (match HW handles
duplicate values in jax tie order — verified on planted duplicates).

Softmax of the 8 selected values stays off the DVE: exp on ScalarE, the
denominator tree-sum and final scale on GPSIMD, reciprocal via the 1-cycle
DVE approx seed + Newton-Raphson refinement on GPSIMD (exact DVE reciprocal
on the final small tile to shorten the serial tail).
"""

import sys

if "/opt/trn_rl_repo" not in sys.path:
    sys.path.insert(0, "/opt/trn_rl_repo")

from dataclasses import dataclass

import numpy as np

N_TOKENS = 1048576
E = 64             # experts
K = 8              # top-k
NCORES = 8
P = 128            # SBUF partitions
TPC = N_TOKENS // NCORES   # tokens per core = 131072
TPP = TPC // P             # tokens per partition = 1024
T = 64                     # tokens per partition per full tile

_CACHE = {}


# --------------------------------------------------------------------------
# Custom fused top-8 DVE op (values + match indices in one instruction).
#
# uOp chain (intra-spec indices; tok_len = 64):
#   0: RAMP0 entry (elem 0 seeds stage-0 swap)   SRC_DONE->IDLE, CNT1->2
#   1: RAMP0 loop  (same config; loop target)    SRC_DONE->IDLE, CNT1->2
#   2..8: RAMP1..7 (MIN cascade j<k + seed k)    CNT1->next
#   9: STEADY_A    (8-stage MIN cascade)         CNT56->10
#  10: CLEAR       (clear_match bubble)          CNT1->11
#  11: STEADY_B    (IS_EQ(stream, swap), latch)  CNT64->12   [reads SRC_1]
#  12: SPACER      (pipeline flush bubble)       CNT1->13
#  13: IDX_DRAIN   (8x OutSel.MATCH_INDEX)       CNT8->14
#  14..21: VDRAIN s0..s7 (descending values)     CNT1->next; last->1 (loop)
# --------------------------------------------------------------------------

def _build_topk_uops(tok_len=64):
    from concourse.dve_uop import (
        AluInp, AluOp, InpSel, OutPath, OutSel, Trigger, UopConfig, ENABLE,
    )

    def ramp(k):
        """Element k of a token: MIN-cascade through stages < k, then a
        swap-EXCHANGE at stage k: BYPASS(a=CURR_SWAP_OUT, b=chain elem) emits
        the PREVIOUS token's (k+1)-th largest (alu_out = a) while latching the
        new seed (swap <- b), so the 8 value drains ride the next token's ramp
        for free. Stages > k forward the emitted value to the write port."""
        u = UopConfig()
        u.enable_input(InpSel.SRC_0, 0)
        u.require_inp0 = ENABLE
        u.repeat_count = 1
        for j in range(k):
            u.datapath_config[j].enable_alu(
                AluOp.MIN, AluInp.CURR_SWAP_OUT, AluInp.PREV_ALU_OUT
            )
            u.datapath_config[j].swap_enable = ENABLE
        u.datapath_config[k].enable_alu(
            AluOp.BYPASS, AluInp.CURR_SWAP_OUT, AluInp.PREV_ALU_OUT
        )
        u.datapath_config[k].swap_enable = ENABLE
        for j in range(k + 1, 8):
            u.datapath_config[j].pass_through_alu()
        u.enable_output(OutSel.ALU_OUT, OutPath.WR0_LO)
        return u

    uops = []
    # Termination: src0's AP carries ONE extra element past the last token, so
    # ramp0 always has data to issue (it never stalls on requires_src0) and
    # the level-evaluated SRC_TENSOR_LT_8 fires on that issue cycle -> IDLE.
    # (Waiting stalled on SRC_TENSOR_DONE after the stream drained misses the
    # done event and leaves the uOp FSM stalled past instruction retirement,
    # wedging the engine for the next NEFF execution.)
    for _ in (0, 1):  # 0: entry, 1: loop re-entry (next_uop 0 means IDLE)
        u = ramp(0)
        # clear_match rides here: fires at this uop's entry, one cycle after
        # the previous token's idx drain issued its last readout, and the
        # latches stay untouched until this token's own match pass
        u.clear_match = ENABLE
        # exit to the epilogue (drain the last token's remaining 7 values)
        u.trigger = (Trigger.SRC_TENSOR_LT_8, Trigger.COUNT, Trigger.NONE)
        u.next_uop = (13, 2, 0)
        uops.append(u)
    for k in range(1, 8):
        u = ramp(k)
        u.trigger = (Trigger.COUNT, Trigger.NONE, Trigger.NONE)
        u.next_uop = (k + 2, 0, 0)
        uops.append(u)

    u = UopConfig()  # 9: steady_A
    u.enable_input(InpSel.SRC_0, 0)
    u.require_inp0 = ENABLE
    u.repeat_count = tok_len - 8
    for j in range(8):
        u.datapath_config[j].enable_alu(
            AluOp.MIN, AluInp.PREV_ALU_OUT, AluInp.CURR_SWAP_OUT
        )
        u.datapath_config[j].swap_enable = ENABLE
    u.trigger = (Trigger.COUNT, Trigger.NONE, Trigger.NONE)
    u.next_uop = (10, 0, 0)
    uops.append(u)

    u = UopConfig()  # 10: steady_B — match pass over the second read port
    u.enable_input(InpSel.SRC_1, 1)  # lane 1 -> delay chain 0
    u.require_inp1 = ENABLE
    u.repeat_count = tok_len
    u.valid_match = ENABLE
    for j in range(8):
        u.datapath_config[j].enable_alu(
            AluOp.IS_EQ, AluInp.PREV_DELAY_0, AluInp.CURR_SWAP_OUT
        )
        u.datapath_config[j].pass_through_delay(0)
    u.trigger = (Trigger.COUNT, Trigger.NONE, Trigger.NONE)
    u.next_uop = (11, 0, 0)
    uops.append(u)

    u = UopConfig()  # 11: spacer — one flush cycle so a match at the LAST
    # stream position settles its latch before the readout samples it
    # (dropping this loses idx[0] whenever the top-1 sits at position 63)
    u.repeat_count = 1
    u.trigger = (Trigger.COUNT, Trigger.NONE, Trigger.NONE)
    u.next_uop = (12, 0, 0)
    uops.append(u)

    u = UopConfig()  # 12: idx drain, then straight to the next token's ramp
    u.repeat_count = 8
    u.enable_output(OutSel.MATCH_INDEX, OutPath.WR0_LO)
    u.trigger = (Trigger.COUNT, Trigger.NONE, Trigger.NONE)
    u.next_uop = (1, 0, 0)
    uops.append(u)

    for k in range(1, 8):  # 13..19: epilogue value drains s1..s7 (last token;
        # its s0 was emitted by the ramp0 issue that took the LT_8 exit)
        u = UopConfig()
        u.repeat_count = 1
        u.datapath_config[k].enable_alu(
            AluOp.BYPASS, AluInp.CURR_SWAP_OUT, AluInp.CURR_SWAP_OUT
        )
        for j in range(k + 1, 8):
            u.datapath_config[j].pass_through_alu()
        u.enable_output(OutSel.ALU_OUT, OutPath.WR0_LO)
        u.trigger = (Trigger.COUNT, Trigger.NONE, Trigger.NONE)
        u.next_uop = (13 + k if k < 7 else 0, 0, 0)
        uops.append(u)
    return uops


def _get_topk_op():
    if "op" in _CACHE:
        return _CACHE["op"]
    from concourse.dve_ops import (
        DveOp, OPS, CUSTOM_DVE_SPECS, _SUB_OPCODE_FOR_NAME, get_dve_sub_opcode,
    )
    from concourse.dve_spec import Spec, Src0, Src1
    from concourse.dve_uop import DveOpSpec

    uops = _build_topk_uops(E)
    # op name carries the uop-bytes hash: a uop edit changes the BIR and so
    # the NEFF cache key, preventing stale-table reuse.
    tag = DveOpSpec(name="probe", opcode=1, uops=uops, rd1_en=True).sha("v3")[:8]
    name = f"TOPK8_{tag}"

    @dataclass(frozen=True)
    class RawDveOp(DveOp):
        raw_uops: tuple = ()

        def compile(self, ver):
            assert ver == "v3", f"hand-written for TRN2/v3 only, got {ver}"
            return DveOpSpec(
                name=self.name,
                opcode=get_dve_sub_opcode(self.name),
                uops=list(self.raw_uops),
                rd1_en=True,
            )

    def _ref(in0, in1, s0, s1, imm2):
        # CoreSim-only; the HW path never calls this.
        p = in0.shape[0]
        x = in0.reshape(p, -1, E)
        t = x.shape[1]
        out = np.zeros((p, t * 16), dtype=np.float32)
        order = np.argsort(-x, axis=-1, kind="stable")[..., :8]
        vals = np.take_along_axis(x, order, axis=-1)
        out.reshape(p, t, 16)[:, :, 0:8] = order.astype(np.uint32).view(np.float32)
        out.reshape(p, t, 16)[:, :, 8:16] = vals
        return out

    op = RawDveOp(
        name=name,
        spec=Spec(body=Src0 + Src1, reference=_ref),
        subdim=False,
        uops_sha={},
        raw_uops=tuple(uops),
    )
    if name not in _SUB_OPCODE_FOR_NAME:
        row = max(_SUB_OPCODE_FOR_NAME.values()) + 1
        assert row < 0x20, f"row {row} overflows the 5-bit byte-36 field"
        OPS.append(op)
        CUSTOM_DVE_SPECS[op.name] = op.spec
        _SUB_OPCODE_FOR_NAME[op.name] = row
    _CACHE["op"] = op
    return op


def _build(tpp=TPP, t_tile=T):
    import concourse.bacc as bacc
    import concourse.mybir as mybir
    import concourse.tile as tile

    f32 = mybir.dt.float32
    op = _get_topk_op()

    n_tok = P * tpp
    # small first tiles (DVE starts after ~512KB of DMA instead of 3MB) and a
    # small last tile (short softmax/store tail after the final DVE instr);
    # 96-token tiles keep the match-index stream under the counter range
    # (96*136 < 16384) while amortizing per-tile semaphore/queue overhead
    if tpp == 1024:
        sizes = [16, 48] + [96] * 9 + [64, 32]
    else:
        sizes = [t_tile] * (tpp // t_tile)
    assert sum(sizes) == tpp
    offs = [sum(sizes[:j]) for j in range(len(sizes))]

    nc = bacc.Bacc("TRN2", target_bir_lowering=False, debug=False)
    logits = nc.dram_tensor("logits", [n_tok, E], f32, kind="ExternalInput")
    # y_out row = [idx0..idx7 (u32 bit patterns), v0..v7]; DMAing the whole
    # tile keeps the store contiguous (one descriptor per partition) — the
    # host reads the idx half and ignores the value half. w is the softmax.
    y_out = nc.dram_tensor("y_out", [n_tok, 16], f32, kind="ExternalOutput")
    w_out = nc.dram_tensor("w_out", [n_tok, K], f32, kind="ExternalOutput")

    lg_v = logits.ap().rearrange("(p t) e -> p t e", p=P, t=tpp)
    y_v = y_out.ap().rearrange("(p t) k -> p t k", p=P, t=tpp)
    w_v = w_out.ap().rearrange("(p t) k -> p t k", p=P, t=tpp)

    with tile.TileContext(nc) as tc:
        with tc.tile_pool(name="io", bufs=5) as pool:

            def softmax_tail(tt, o, y, ex):
                """Softmax on the DVE (reduce + reciprocal + scale). The DVE
                custom op's dual-stream SBUF traffic starves the GPSIMD Q7s
                (~10x slowdown while it runs), so the softmax runs on the DVE
                queue itself; emitted one tile late, the inputs are always
                ready and these ~1.3us never stall the queue."""
                s = pool.tile([P, tt, 1], f32, tag="s")
                nc.vector.tensor_reduce(
                    s[:], ex[:], axis=mybir.AxisListType.X,
                    op=mybir.AluOpType.add,
                )
                r = pool.tile([P, tt, 1], f32, tag="r")
                # ~51-ULP single-pass approx (exact divide iterates 8 cycles
                # per element); 4e-6 relative on w, far inside the tolerance
                nc.vector.reciprocal_approx_fast(r[:], s[:])
                # the broadcast-scale runs on GPSIMD: the Q7s crawl while the
                # custom DVE op streams (SBUF port starvation), but with ~3
                # tiles of slack before the w DMA needs it, even a starved
                # multiply is far off the critical path — and it keeps ~1us
                # per tile off the serial DVE queue
                w = pool.tile([P, tt, K], f32, tag="w")
                nc.gpsimd.tensor_mul(w[:], ex[:], r[:].broadcast_to([P, tt, K]))
                # output DMAs issue from the Pool queue: the serial Sync queue
                # carries ONLY input loads, so the next tile's x transfer is
                # never queued behind output DMAs waiting on the softmax.
                nc.gpsimd.dma_start(y_v[:, o:o + tt, :], y[:])
                nc.gpsimd.dma_start(w_v[:, o:o + tt, :], w[:])

            prev = None
            for o, tt in zip(offs, sizes):
                # one extra trailing element feeds the LT_8 termination issue
                x = pool.tile([P, tt * E + 1], f32, tag="x")
                x3 = x[:, 0:tt * E].rearrange("p (t e) -> p t e", t=tt, e=E)
                nc.sync.dma_start(x3, lg_v[:, o:o + tt, :])
                # output stream: 8 garbage words (stale swap flops emitted by
                # the first token's ramp), then per token [idx x8, vals x8]
                yr = pool.tile([P, tt * 16 + 8], f32, tag="y")
                y = yr[:, 8:].rearrange("p (t k) -> p t k", t=tt, k=16)
                nc.vector._custom_dve(op, out=yr[:], in0=x[:], in1=x3,
                                      s0=0.0, s1=0.0)
                ex = pool.tile([P, tt, K], f32, tag="ex")
                nc.scalar.activation(
                    ex[:], y[:, :, 8:16], mybir.ActivationFunctionType.Exp
                )
                if prev is not None:
                    softmax_tail(*prev)
                prev = (tt, o, y, ex)
            softmax_tail(*prev)
    nc.compile()
    return nc


def _get_nc():
    if "nc" not in _CACHE:
        _CACHE["nc"] = _build()
    return _CACHE["nc"]


def kernel(logits, top_k):
    logits = np.asarray(logits, dtype=np.float32)
    k = int(np.asarray(top_k))
    assert k == K, f"kernel hardcodes top_k={K}, got {k}"
    assert logits.shape == (N_TOKENS, E), logits.shape

    from concourse.bass_utils import run_bass_kernel_spmd

    nc = _get_nc()
    chunks = logits.reshape(NCORES, TPC, E)
    in_maps = [{"logits": np.ascontiguousarray(chunks[c])} for c in range(NCORES)]
    # The tunneled devices occasionally fail a run with a transient
    # NRT_EXEC_UNIT_UNRECOVERABLE error; a straight retry recovers.
    last_err = None
    for _attempt in range(3):
        try:
            res = run_bass_kernel_spmd(nc, in_maps, list(range(NCORES)))
            break
        except Exception as e:  # noqa: BLE001 - retry transient device faults
            last_err = e
            import time as _time

            _time.sleep(5.0)
    else:
        raise last_err

    # Row r of each per-core output is token r of that core's slice, so a
    # plain concat along the token axis reassembles the full outputs.
    y = np.concatenate([r["y_out"] for r in res.results], axis=0)
    w = np.concatenate([r["w_out"] for r in res.results], axis=0)
    idx = np.ascontiguousarray(y[:, 0:K]).view(np.uint32).astype(np.int64)
    return idx, w.astype(np.float32)
